# revision 15
# baseline (speedup 1.0000x reference)
"""Trainium2 Bass/Tile kernel for nn_Block_40475771797636 (trajectory-attention
transformer block), 8-way data/sequence parallel.

Sharding: cores 0-3 take batch 0, cores 4-7 take batch 1. Within a batch each
core owns 392 query tokens (= 2 frames). K/V for the whole batch are computed
on every core (duplicated, cheap); everything else is sharded by query token.

Per-core inputs are frame-PERMUTED on the host so the same SPMD program works
on every core: the core's own 2 query frames are always frames 0,1 of its
input. All cross-frame ops (stage-2 softmax over frames, output sum over
frames) are frame-order invariant, and the cls token attends to all tokens
(order invariant), so the permutation changes nothing except which rows the
core's outputs correspond to.

Layout strategy: activations are kept FEATURE-major on chip ([C on partitions,
tokens on free]) so chained matmuls need no transposes; LN is done token-major
(free-axis reductions) with PE transposes between. Stage-1 attention computes
scores TRANSPOSED ([keys, queries]) so the softmax denominator comes from a
ones-matmul and the attn@V product needs no transpose of the attention matrix.

Note: the reference computes kv2 = x1 @ Wkv but only uses the k half (v2 is
dead code), so only Wkv[:, :C] is used here.
"""

import sys

import numpy as np

if "/opt/trn_rl_repo" not in sys.path:
    sys.path.insert(0, "/opt/trn_rl_repo")

import concourse.bass as bass
from concourse import bacc
import concourse.mybir as mybir
import concourse.tile as tile
from contextlib import ExitStack

F32 = mybir.dt.float32
F32R = mybir.dt.float32r
AF = mybir.ActivationFunctionType
ALU = mybir.AluOpType

C = 768
CO = 6          # C / 128 contraction chunks
H = 12
NPAIR = 6       # head pairs
D = 64
F = 8           # frames
P = 196         # patches per frame
S = F * P       # 1568 non-cls tokens
N = 1 + S       # 1569
Q = 392         # queries per core (2 frames)
NQ = 1 + Q      # cls + queries
NQP = NQ + 1    # NQ padded even (fp32r matmul dst needs even free count)
NP2 = N + 1     # N padded even
HID = 4 * C
HO = 24         # HID / 128
SCALE = 0.125   # d ** -0.5
EPS = 1e-5

# token chunks used for token-major V and for partition-chunked attention keys:
# per frame two chunks (128 + 68 patches), cls token as final chunk idx 16
VCHUNKS = []
for _f in range(F):
    VCHUNKS.append((1 + P * _f, 128))
    VCHUNKS.append((1 + P * _f + 128, 68))
VCHUNKS.append((0, 1))
NCH = len(VCHUNKS)  # 17

# token tiles for K-producing matmuls (free dim <= 512)
KTOKS = [(0, 512), (512, 512), (1024, 512), (1536, NP2 - 1536)]
# token-major tiles covering the NQ output rows
TT4 = [(0, 128), (128, 128), (256, 128), (384, NQ - 384)]
# token-major tiles covering all N rows (for LN1)
TTN = [(t * 128, min(128, N - t * 128)) for t in range((N + 127) // 128)]


def _bcast_ap(vec_ap, parts=128):
    """DMA access pattern replicating a 1-D dram vector across partitions."""
    return bass.AP(
        tensor=vec_ap.tensor,
        offset=vec_ap.offset,
        ap=[[0, parts]] + [list(a) for a in vec_ap.ap],
    )


def _vseg(vsb, ch, cn, jl, hl):
    """128-wide contiguous [v | 0] / [0 | v] stationary operand for head
    (pair jl, half hl): vsb stores per pair [v_even(64) | zeros(64) |
    v_odd(64)] so a 64-row result lands in either psum half without column
    tiling (which fp32r rejects)."""
    base = 192 * jl
    if hl == 0:
        return vsb[:cn, ch, base : base + 128]
    return vsb[:cn, ch, base + 64 : base + 192]


def _oseg(onez, cn, hl):
    """Same trick for the softmax-denominator ones vector."""
    if hl == 0:
        return onez[:cn, 64:192]
    return onez[:cn, 0:128]


def _mm(nc, out, lhsT, rhs, first=True, last=True, tile_position=None):
    nc.tensor.matmul(
        out,
        lhsT.bitcast(F32R),
        rhs.bitcast(F32R),
        start=first,
        stop=last,
        tile_position=tile_position,
    )


def _layernorm_tile(nc, pools, xt, pt, g_rep, b_rep, eps_t, out_t):
    """Token-major LN of xt[:pt, :C] into out_t[:pt, :C]."""
    stats = pools.tile([128, 3, 6], F32, tag="ln_stats")
    xv = xt[:pt].rearrange("p (s q) -> p s q", s=3)
    for sg in range(3):
        nc.vector.bn_stats(out=stats[:pt, sg], in_=xv[:, sg])
    mv = pools.tile([128, 2], F32, tag="ln_mv")
    nc.vector.bn_aggr(out=mv[:pt], in_=stats[:pt])
    rs = pools.tile([128, 1], F32, tag="ln_rs")
    nc.scalar.activation(
        out=rs[:pt], in_=mv[:pt, 1:2], func=AF.Sqrt, bias=eps_t[:pt], scale=1.0
    )
    nc.vector.reciprocal(out=rs[:pt], in_=rs[:pt])
    nc.vector.tensor_scalar(
        out_t[:pt],
        xt[:pt],
        mv[:pt, 0:1],
        rs[:pt],
        ALU.subtract,
        ALU.mult,
    )
    nc.vector.tensor_tensor(out=out_t[:pt], in0=out_t[:pt], in1=g_rep[:pt], op=ALU.mult)
    nc.vector.tensor_tensor(out=out_t[:pt], in0=out_t[:pt], in1=b_rep[:pt], op=ALU.add)


def build_nc():
    nc = bacc.Bacc("TRN2", target_bir_lowering=False)

    xp_d = nc.dram_tensor("xp", (N, C), F32, kind="ExternalInput")
    wqkv_d = nc.dram_tensor("wqkv", (C, 3 * C), F32, kind="ExternalInput")
    wq_d = nc.dram_tensor("wq", (C, C), F32, kind="ExternalInput")
    wk2_d = nc.dram_tensor("wk2", (C, C), F32, kind="ExternalInput")
    wp_d = nc.dram_tensor("wp", (C, C), F32, kind="ExternalInput")
    w1_d = nc.dram_tensor("w1", (C, HID), F32, kind="ExternalInput")
    w2_d = nc.dram_tensor("w2", (HID, C), F32, kind="ExternalInput")
    g1_d = nc.dram_tensor("g1", (C,), F32, kind="ExternalInput")
    b1_d = nc.dram_tensor("b1", (C,), F32, kind="ExternalInput")
    g2_d = nc.dram_tensor("g2", (C,), F32, kind="ExternalInput")
    b2_d = nc.dram_tensor("b2", (C,), F32, kind="ExternalInput")
    bp_d = nc.dram_tensor("bp", (C,), F32, kind="ExternalInput")
    bf1_d = nc.dram_tensor("bf1", (HID,), F32, kind="ExternalInput")
    bf2_d = nc.dram_tensor("bf2", (C,), F32, kind="ExternalInput")
    ident_d = nc.dram_tensor("ident", (128, 128), F32, kind="ExternalInput")
    onez_d = nc.dram_tensor("onez", (128, 192), F32, kind="ExternalInput")
    e2_d = nc.dram_tensor("e2", (128, 128), F32, kind="ExternalInput")
    out_d = nc.dram_tensor("out", (NQ, C), F32, kind="ExternalOutput")

    wqkv = wqkv_d[:].rearrange("(o p) n -> p o n", p=128)
    wq_fm = wq_d[:].rearrange("(o p) n -> p o n", p=128)
    wk2_fm = wk2_d[:].rearrange("(o p) n -> p o n", p=128)
    wp_fm = wp_d[:].rearrange("(o p) n -> p o n", p=128)
    w1_fm = w1_d[:].rearrange("(o p) n -> p o n", p=128)
    w2_fm = w2_d[:].rearrange("(m p) n -> p m n", p=128)

    with tile.TileContext(nc) as tc, ExitStack() as root:
        consts = root.enter_context(tc.tile_pool(name="consts", bufs=1))
        outp = root.enter_context(tc.tile_pool(name="outp", bufs=1))

        ident = consts.tile([128, 128], F32)
        nc.gpsimd.dma_start(out=ident, in_=ident_d[:])
        onez = consts.tile([128, 192], F32R)
        nc.gpsimd.dma_start(out=onez, in_=onez_d[:].bitcast(F32R))
        e2 = consts.tile([128, 128], F32R)
        nc.gpsimd.dma_start(out=e2, in_=e2_d[:].bitcast(F32R))
        bp_sb = consts.tile([128, CO], F32)
        nc.gpsimd.dma_start(out=bp_sb, in_=bp_d[:].rearrange("(o p) -> p o", p=128))
        bf1_sb = consts.tile([128, HO], F32)
        nc.gpsimd.dma_start(out=bf1_sb, in_=bf1_d[:].rearrange("(o p) -> p o", p=128))
        bf2_sb = consts.tile([128, CO], F32)
        nc.gpsimd.dma_start(out=bf2_sb, in_=bf2_d[:].rearrange("(o p) -> p o", p=128))
        eps_t = consts.tile([128, 1], F32)
        nc.vector.memset(eps_t, EPS)

        # attention block output (feature-major): col 0 = cls, cols 1.. = queries
        outT = outp.tile([128, NPAIR, NQP], F32R)

        with ExitStack() as s_attn:
            xnTp = s_attn.enter_context(tc.tile_pool(name="xnTp", bufs=1))
            xnT = xnTp.tile([128, CO, NP2], F32R)

            # ---------- Phase A1: LN1 (token-major) + transpose to xnT ----------
            with ExitStack() as ph:
                xio = ph.enter_context(tc.tile_pool(name="xio", bufs=3))
                lnp = ph.enter_context(tc.tile_pool(name="lnp", bufs=4))
                gbp = ph.enter_context(tc.tile_pool(name="gbp", bufs=1))
                tps = ph.enter_context(tc.tile_pool(name="tps", bufs=4, space="PSUM"))
                g1r = gbp.tile([128, C], F32, tag="g1r")
                nc.gpsimd.dma_start(out=g1r, in_=_bcast_ap(g1_d[:]))
                b1r = gbp.tile([128, C], F32, tag="b1r")
                nc.gpsimd.dma_start(out=b1r, in_=_bcast_ap(b1_d[:]))
                for t0, pt in TTN:
                    xt = xio.tile([128, C], F32, tag="xt")
                    nc.gpsimd.dma_start(out=xt[:pt], in_=xp_d[t0 : t0 + pt])
                    xn = xio.tile([128, C], F32, tag="xn")
                    _layernorm_tile(nc, lnp, xt, pt, g1r, b1r, eps_t, xn)
                    for o in range(CO):
                        pst = tps.tile([128, 128], F32, tag="tp")
                        nc.tensor.transpose(
                            pst[:, :pt],
                            xn[:pt, o * 128 : (o + 1) * 128],
                            ident[:pt, :pt],
                        )
                        nc.any.tensor_copy(
                            out=xnT[:, o, t0 : t0 + pt], in_=pst[:, :pt]
                        )

            # ---------- head-group loop: QKV + stage-1 attention + cls ----------
            with ExitStack() as s_x1:
                x1Tp = s_x1.enter_context(tc.tile_pool(name="x1Tp", bufs=1))
                x1T = x1Tp.tile([128, NPAIR, F, Q], F32R)

                for g in range(2):
                    pairs = [3 * g, 3 * g + 1, 3 * g + 2]
                    with ExitStack() as sg:
                        kqvp = sg.enter_context(tc.tile_pool(name="kqv", bufs=1))
                        kT = kqvp.tile([128, 3, NP2], F32R, tag="kT")
                        qT = kqvp.tile([128, 3, NQP], F32R, tag="qT")
                        vsb = kqvp.tile([128, NCH, 576], F32R, tag="vsb")
                        for _ch in range(NCH):
                            for _jl in range(3):
                                nc.vector.memset(
                                    vsb[
                                        :, _ch, 192 * _jl + 64 : 192 * _jl + 128
                                    ].bitcast(F32),
                                    0.0,
                                )

                        # ----- A-g: K/Q (feature-major) and V (token-major) -----
                        with ExitStack() as sa:
                            wkq = sa.enter_context(tc.tile_pool(name="wkq", bufs=2))
                            wvv = sa.enter_context(tc.tile_pool(name="wvv", bufs=1))
                            kqps = sa.enter_context(
                                tc.tile_pool(name="kqps", bufs=2, space="PSUM")
                            )
                            vps = sa.enter_context(
                                tc.tile_pool(name="vps", bufs=4, space="PSUM")
                            )
                            for jl, j in enumerate(pairs):
                                wk_t = wkq.tile([128, CO, 128], F32R, tag="wk")
                                nc.gpsimd.dma_start(
                                    out=wk_t,
                                    in_=wqkv[:, :, C + j * 128 : C + (j + 1) * 128].bitcast(F32R),
                                )
                                for tt0, tn in KTOKS:
                                    ps = kqps.tile([128, 512], F32, tag="kqps")
                                    for o in range(CO):
                                        _mm(
                                            nc,
                                            ps[:, :tn],
                                            wk_t[:, o],
                                            xnT[:, o, tt0 : tt0 + tn],
                                            o == 0,
                                            o == CO - 1,
                                        )
                                    nc.any.tensor_copy(
                                        out=kT[:, jl, tt0 : tt0 + tn], in_=ps[:, :tn]
                                    )
                                wq_t = wkq.tile([128, CO, 128], F32R, tag="wqg")
                                nc.gpsimd.dma_start(
                                    out=wq_t, in_=wqkv[:, :, j * 128 : (j + 1) * 128].bitcast(F32R)
                                )
                                ps = kqps.tile([128, 512], F32, tag="kqps")
                                for o in range(CO):
                                    _mm(
                                        nc,
                                        ps[:, :NQP],
                                        wq_t[:, o],
                                        xnT[:, o, 0:NQP],
                                        o == 0,
                                        o == CO - 1,
                                    )
                                nc.any.tensor_copy(out=qT[:, jl], in_=ps[:, :NQP])
                            # V for this head group (6 heads = 384 cols)
                            wv_t = wvv.tile([128, CO, 384], F32R, tag="wv")
                            nc.gpsimd.dma_start(
                                out=wv_t,
                                in_=wqkv[:, :, 2 * C + g * 384 : 2 * C + (g + 1) * 384].bitcast(F32R),
                            )
                            for ch, (c0, cn) in enumerate(VCHUNKS):
                                ps = vps.tile([128, 384], F32, tag="vps")
                                for o in range(CO):
                                    _mm(
                                        nc,
                                        ps[:cn],
                                        xnT[:, o, c0 : c0 + cn],
                                        wv_t[:, o],
                                        o == 0,
                                        o == CO - 1,
                                    )
                                vv = vsb[:cn, ch].rearrange("p (j s) -> p j s", s=192)
                                pv = ps[:cn].rearrange("p (j h e) -> p j h e", j=3, h=2)
                                nc.any.tensor_copy(out=vv[:, :, 0:64], in_=pv[:, :, 0])
                                nc.any.tensor_copy(
                                    out=vv[:, :, 128:192], in_=pv[:, :, 1]
                                )

                        # ----- B-g: stage-1 trajectory attention -----
                        with ExitStack() as sb:
                            scps = sb.enter_context(
                                tc.tile_pool(name="scps", bufs=1, space="PSUM")
                            )
                            sxps = sb.enter_context(
                                tc.tile_pool(name="sxps", bufs=2, space="PSUM")
                            )
                            ssps = sb.enter_context(
                                tc.tile_pool(name="ssps", bufs=2, space="PSUM")
                            )
                            expp = sb.enter_context(tc.tile_pool(name="expp", bufs=3))
                            rcpp = sb.enter_context(tc.tile_pool(name="rcpp", bufs=2))
                            for jl, j in enumerate(pairs):
                                for fg in range(4):  # 2 frames per group
                                    ext = []
                                    for hl in range(2):
                                        dlo = hl * 64
                                        sps = scps.tile([128, 4, 512], F32, tag="sc")
                                        for fi in range(2):
                                            f = fg * 2 + fi
                                            for ci in range(2):
                                                c0, cn = VCHUNKS[2 * f + ci]
                                                _mm(
                                                    nc,
                                                    sps[:cn, 2 * fi + ci, :Q],
                                                    kT[dlo : dlo + 64, jl, c0 : c0 + cn],
                                                    qT[dlo : dlo + 64, jl, 1:NQ],
                                                )
                                        ex = expp.tile([128, 4, Q], F32R, tag="ex")
                                        nc.scalar.activation(
                                            out=ex,
                                            in_=sps[:, :, :Q],
                                            func=AF.Exp,
                                            scale=SCALE,
                                        )
                                        ext.append(ex)
                                    for fi in range(2):
                                        f = fg * 2 + fi
                                        sx = sxps.tile([128, Q], F32, tag="sx")
                                        ss = ssps.tile([128, Q], F32, tag="ss")
                                        for hl in range(2):
                                            gh = jl * 2 + hl
                                            for ci in range(2):
                                                c0, cn = VCHUNKS[2 * f + ci]
                                                exc = ext[hl][:cn, 2 * fi + ci]
                                                first = hl == 0 and ci == 0
                                                last = hl == 1 and ci == 1
                                                _mm(
                                                    nc,
                                                    sx,
                                                    _vseg(vsb, 2 * f + ci, cn, jl, hl),
                                                    exc,
                                                    first,
                                                    last,
                                                )
                                                _mm(
                                                    nc,
                                                    ss,
                                                    _oseg(onez, cn, hl),
                                                    exc,
                                                    first,
                                                    last,
                                                )
                                        rc = rcpp.tile([128, Q], F32, tag="rc")
                                        nc.vector.reciprocal(out=rc, in_=ss)
                                        nc.vector.tensor_tensor(
                                            out=x1T[:, j, f],
                                            in0=sx,
                                            in1=rc,
                                            op=ALU.mult,
                                        )

                        # ----- B5-g: cls attention (over all tokens) -----
                        with ExitStack() as s5:
                            cps = s5.enter_context(
                                tc.tile_pool(name="cps", bufs=2, space="PSUM")
                            )
                            czps = s5.enter_context(
                                tc.tile_pool(name="czps", bufs=2, space="PSUM")
                            )
                            cops = s5.enter_context(
                                tc.tile_pool(name="cops", bufs=2, space="PSUM")
                            )
                            cexp = s5.enter_context(tc.tile_pool(name="cexp", bufs=2))
                            czrp = s5.enter_context(tc.tile_pool(name="czrp", bufs=2))
                            for jl, j in enumerate(pairs):
                                # column 0 = cls query; column 1 = padding (the
                                # first real query) so fp32r dst free counts
                                # stay even
                                co_ps = cops.tile([128, 2], F32, tag="co")
                                zps = czps.tile([128, NCH, 2], F32, tag="cz")
                                zrr = czrp.tile([128, 1], F32, tag="zr")
                                exs = []
                                for hl in range(2):
                                    dlo = hl * 64
                                    sps = cps.tile([128, NCH, 2], F32, tag="cs")
                                    for ch, (c0, cn) in enumerate(VCHUNKS):
                                        _mm(
                                            nc,
                                            sps[:cn, ch],
                                            kT[dlo : dlo + 64, jl, c0 : c0 + cn],
                                            qT[dlo : dlo + 64, jl, 0:2],
                                        )
                                    ex = cexp.tile([128, NCH, 2], F32R, tag="cex")
                                    nc.scalar.activation(
                                        out=ex, in_=sps, func=AF.Exp, scale=SCALE
                                    )
                                    exs.append(ex)
                                # per-chunk partition-restricted sums so junk
                                # lanes (partitions past chunk size) are never read
                                for ch, (c0, cn) in enumerate(VCHUNKS):
                                    for hl in range(2):
                                        _mm(
                                            nc,
                                            zps[:, ch],
                                            _oseg(onez, cn, hl),
                                            exs[hl][:cn, ch],
                                            hl == 0,
                                            hl == 1,
                                        )
                                nc.vector.reduce_sum(
                                    out=zrr,
                                    in_=zps[:, :, 0],
                                    axis=mybir.AxisListType.X,
                                )
                                nc.vector.reciprocal(out=zrr, in_=zrr)
                                for hl in range(2):
                                    gh = jl * 2 + hl
                                    for ch, (c0, cn) in enumerate(VCHUNKS):
                                        _mm(
                                            nc,
                                            co_ps,
                                            _vseg(vsb, ch, cn, jl, hl),
                                            exs[hl][:cn, ch],
                                            hl == 0 and ch == 0,
                                            hl == 1 and ch == NCH - 1,
                                        )
                                nc.vector.tensor_scalar_mul(
                                    outT[:, j, 0:1], co_ps[:, 0:1], zrr
                                )

                # ---------- Phase C: stage-2 frame attention ----------
                with ExitStack() as sC:
                    xdp = sC.enter_context(tc.tile_pool(name="xdp", bufs=1))
                    q2p = sC.enter_context(tc.tile_pool(name="q2p", bufs=1))
                    wsC = sC.enter_context(tc.tile_pool(name="wsC", bufs=2))
                    k2ps = sC.enter_context(
                        tc.tile_pool(name="k2ps", bufs=3, space="PSUM")
                    )
                    lps = sC.enter_context(tc.tile_pool(name="lps", bufs=2, space="PSUM"))
                    prp = sC.enter_context(tc.tile_pool(name="prp", bufs=3))
                    elp = sC.enter_context(tc.tile_pool(name="elp", bufs=2))
                    zzp = sC.enter_context(tc.tile_pool(name="zzp", bufs=2))

                    # x_diag: query i<196 -> frame 0, else frame 1 (frame-permuted)
                    xdT = xdp.tile([128, CO, Q], F32R)
                    nc.any.tensor_copy(out=xdT[:, :, 0:P], in_=x1T[:, :, 0, 0:P])
                    nc.any.tensor_copy(out=xdT[:, :, P:Q], in_=x1T[:, :, 1, P:Q])

                    q2T = q2p.tile([128, CO, Q], F32)
                    for j in range(NPAIR):
                        wq_t = wsC.tile([128, CO, 128], F32R, tag="wqC")
                        nc.gpsimd.dma_start(
                            out=wq_t, in_=wq_fm[:, :, j * 128 : (j + 1) * 128].bitcast(F32R)
                        )
                        ps = k2ps.tile([128, Q], F32, tag="k2")
                        for o in range(CO):
                            _mm(nc, ps, wq_t[:, o], xdT[:, o], o == 0, o == CO - 1)
                        nc.any.tensor_copy(out=q2T[:, j], in_=ps)

                    for j in range(NPAIR):
                        wk_t = wsC.tile([128, CO, 128], F32R, tag="wkC")
                        nc.gpsimd.dma_start(
                            out=wk_t, in_=wk2_fm[:, :, j * 128 : (j + 1) * 128].bitcast(F32R)
                        )
                        el = elp.tile([128, F, Q], F32, tag="el")
                        for f in range(F):
                            ps = k2ps.tile([128, Q], F32, tag="k2")
                            for o in range(CO):
                                _mm(
                                    nc,
                                    ps,
                                    wk_t[:, o],
                                    x1T[:, o, f],
                                    o == 0,
                                    o == CO - 1,
                                )
                            pr = prp.tile([128, Q], F32R, tag="pr")
                            nc.vector.tensor_tensor(
                                out=pr, in0=ps, in1=q2T[:, j], op=ALU.mult
                            )
                            lp = lps.tile([128, Q], F32, tag="lg")
                            _mm(nc, lp, e2, pr)
                            nc.scalar.activation(
                                out=el[:, f], in_=lp, func=AF.Exp, scale=SCALE
                            )
                        zz = zzp.tile([128, Q], F32, tag="zz")
                        nc.vector.tensor_tensor(
                            out=zz, in0=el[:, 0], in1=el[:, 1], op=ALU.add
                        )
                        for f in range(2, F):
                            nc.vector.tensor_tensor(
                                out=zz, in0=zz, in1=el[:, f], op=ALU.add
                            )
                        nc.vector.reciprocal(out=zz, in_=zz)
                        acc = outT[:, j, 1:NQ]
                        nc.vector.tensor_tensor(
                            out=acc, in0=x1T[:, j, 0], in1=el[:, 0], op=ALU.mult
                        )
                        for f in range(1, F):
                            tm = prp.tile([128, Q], F32, tag="tm")
                            nc.vector.tensor_tensor(
                                out=tm, in0=x1T[:, j, f], in1=el[:, f], op=ALU.mult
                            )
                            nc.vector.tensor_tensor(
                                out=acc, in0=acc, in1=tm, op=ALU.add
                            )
                        nc.vector.tensor_tensor(out=acc, in0=acc, in1=zz, op=ALU.mult)

        # ---------- Phase D: proj + residual + LN2 + MLP + output ----------
        with ExitStack() as sD:
            wpp = sD.enter_context(tc.tile_pool(name="wpD", bufs=2))
            w2p = sD.enter_context(tc.tile_pool(name="w2D", bufs=2))
            xop = sD.enter_context(tc.tile_pool(name="xoD", bufs=1))
            pjp = sD.enter_context(tc.tile_pool(name="pjD", bufs=1))
            h1p = sD.enter_context(tc.tile_pool(name="h1D", bufs=1))
            m2p = sD.enter_context(tc.tile_pool(name="m2D", bufs=1))
            xn2p = sD.enter_context(tc.tile_pool(name="xn2D", bufs=1))
            gbD = sD.enter_context(tc.tile_pool(name="gbD", bufs=1))
            finp = sD.enter_context(tc.tile_pool(name="finp", bufs=2))
            ln2p = sD.enter_context(tc.tile_pool(name="ln2p", bufs=4))
            mps = sD.enter_context(tc.tile_pool(name="mps", bufs=2, space="PSUM"))
            tps2 = sD.enter_context(tc.tile_pool(name="tps2", bufs=4, space="PSUM"))

            g2r = gbD.tile([128, C], F32, tag="g2r")
            nc.gpsimd.dma_start(out=g2r, in_=_bcast_ap(g2_d[:]))
            b2r = gbD.tile([128, C], F32, tag="b2r")
            nc.gpsimd.dma_start(out=b2r, in_=_bcast_ap(b2_d[:]))

            xown = xop.tile([128, 4, C], F32, tag="xo")
            for tt, (t0, pt) in enumerate(TT4):
                nc.gpsimd.dma_start(out=xown[:pt, tt], in_=xp_d[t0 : t0 + pt])

            projT = pjp.tile([128, CO, NQ], F32)
            for j in range(CO):
                wp_t = wpp.tile([128, CO, 128], F32R, tag="wpD")
                nc.gpsimd.dma_start(out=wp_t, in_=wp_fm[:, :, j * 128 : (j + 1) * 128].bitcast(F32R))
                ps = mps.tile([128, 512], F32, tag="mm")
                for o in range(CO):
                    _mm(nc, ps[:, :NQP], wp_t[:, o], outT[:, o], o == 0, o == CO - 1)
                nc.vector.tensor_scalar(
                    projT[:, j], ps[:, :NQ], bp_sb[:, j : j + 1], None, ALU.add
                )

            # residual + LN2 (token-major), then transpose to xn2T
            x2 = xop.tile([128, 4, C], F32, tag="x2")
            xn2T = xn2p.tile([128, CO, NQP], F32R)
            for tt, (t0, pt) in enumerate(TT4):
                for o in range(CO):
                    pst = tps2.tile([128, 128], F32, tag="tp2")
                    nc.tensor.transpose(
                        pst[:pt], projT[:, o, t0 : t0 + pt], ident
                    )
                    nc.vector.tensor_tensor(
                        out=x2[:pt, tt, o * 128 : (o + 1) * 128],
                        in0=pst[:pt],
                        in1=xown[:pt, tt, o * 128 : (o + 1) * 128],
                        op=ALU.add,
                    )
                xn2 = finp.tile([128, C], F32, tag="xn2")
                _layernorm_tile(nc, ln2p, x2[:, tt], pt, g2r, b2r, eps_t, xn2)
                for o in range(CO):
                    pst = tps2.tile([128, 128], F32, tag="tp2")
                    nc.tensor.transpose(
                        pst[:, :pt], xn2[:pt, o * 128 : (o + 1) * 128], ident[:pt, :pt]
                    )
                    nc.any.tensor_copy(out=xn2T[:, o, t0 : t0 + pt], in_=pst[:, :pt])

            # MLP up + gelu
            h1T = h1p.tile([128, HO, NQP], F32R)
            for m in range(HO):
                w1_t = wpp.tile([128, CO, 128], F32R, tag="w1D")
                nc.gpsimd.dma_start(out=w1_t, in_=w1_fm[:, :, m * 128 : (m + 1) * 128].bitcast(F32R))
                ps = mps.tile([128, 512], F32, tag="mm")
                for o in range(CO):
                    _mm(nc, ps[:, :NQP], w1_t[:, o], xn2T[:, o], o == 0, o == CO - 1)
                nc.scalar.activation(
                    out=h1T[:, m, :NQ],
                    in_=ps[:, :NQ],
                    func=AF.Gelu,
                    bias=bf1_sb[:, m : m + 1],
                    scale=1.0,
                )

            # MLP down
            m2T = m2p.tile([128, CO, NQ], F32)
            for j in range(CO):
                w2_t = w2p.tile([128, HO, 128], F32R, tag="w2D")
                nc.gpsimd.dma_start(out=w2_t, in_=w2_fm[:, :, j * 128 : (j + 1) * 128].bitcast(F32R))
                ps = mps.tile([128, 512], F32, tag="mm")
                for m in range(HO):
                    _mm(nc, ps[:, :NQP], w2_t[:, m], h1T[:, m], m == 0, m == HO - 1)
                nc.vector.tensor_scalar(
                    m2T[:, j], ps[:, :NQ], bf2_sb[:, j : j + 1], None, ALU.add
                )

            # transpose back to token-major, final residual, store
            for tt, (t0, pt) in enumerate(TT4):
                fo = finp.tile([128, C], F32, tag="fo")
                for o in range(CO):
                    pst = tps2.tile([128, 128], F32, tag="tp2")
                    nc.tensor.transpose(pst[:pt], m2T[:, o, t0 : t0 + pt], ident)
                    nc.vector.tensor_tensor(
                        out=fo[:pt, o * 128 : (o + 1) * 128],
                        in0=pst[:pt],
                        in1=x2[:pt, tt, o * 128 : (o + 1) * 128],
                        op=ALU.add,
                    )
                nc.gpsimd.dma_start(out=out_d[t0 : t0 + pt], in_=fo[:pt])

    nc.compile()
    return nc


_CACHE = {}


def _get_nc():
    if "nc" not in _CACHE:
        _CACHE["nc"] = build_nc()
    return _CACHE["nc"]


def _make_in_maps(inputs):
    f32 = np.float32
    x = np.asarray(inputs["x"], f32)
    wqkv = np.ascontiguousarray(np.asarray(inputs["Wqkv"], f32))
    wq = np.ascontiguousarray(np.asarray(inputs["Wq"], f32))
    wk2 = np.ascontiguousarray(np.asarray(inputs["Wkv"], f32)[:, :C])
    wp = np.ascontiguousarray(np.asarray(inputs["Wp"], f32))
    w1 = np.ascontiguousarray(np.asarray(inputs["W1"], f32))
    w2 = np.ascontiguousarray(np.asarray(inputs["W2"], f32))
    ident = np.eye(128, dtype=f32)
    onez = np.zeros((128, 192), dtype=f32)
    onez[:, 64:128] = 1.0
    e2 = np.zeros((128, 128), dtype=f32)
    e2[:64, :64] = 1.0
    e2[64:, 64:] = 1.0
    common = dict(
        wqkv=wqkv,
        wq=wq,
        wk2=wk2,
        wp=wp,
        w1=w1,
        w2=w2,
        g1=np.asarray(inputs["g1"], f32),
        b1=np.asarray(inputs["b1"], f32),
        g2=np.asarray(inputs["g2"], f32),
        b2=np.asarray(inputs["b2"], f32),
        bp=np.asarray(inputs["bp"], f32),
        bf1=np.asarray(inputs["bf1"], f32),
        bf2=np.asarray(inputs["bf2"], f32),
        ident=ident,
        onez=onez,
        e2=e2,
    )
    in_maps = []
    for c in range(8):
        b, cl = c // 4, c % 4
        f0 = 2 * cl
        order = [(f0 + i) % F for i in range(F)]
        xb = x[b]
        xp = np.concatenate(
            [xb[:1], xb[1:].reshape(F, P, C)[order].reshape(S, C)], axis=0
        )
        m = dict(common)
        m["xp"] = np.ascontiguousarray(xp)
        in_maps.append(m)
    return in_maps


def kernel(**inputs):
    from concourse.bass_utils import run_bass_kernel_spmd

    in_maps = _make_in_maps(inputs)
    res = run_bass_kernel_spmd(_get_nc(), in_maps, core_ids=list(range(8)))
    outs = res.results
    x = np.asarray(inputs["x"])
    full = np.empty((x.shape[0], N, C), dtype=np.float32)
    for c in range(8):
        r = outs[c]["out"]
        b, cl = c // 4, c % 4
        if cl == 0:
            full[b, 0] = r[0]
        full[b, 1 + Q * cl : 1 + Q * (cl + 1)] = r[1:]
    return full


# revision 18
# speedup vs baseline: 3.3911x; 3.3911x over previous
"""Trainium2 Bass/Tile kernel for nn_Block_40475771797636 (trajectory-attention
transformer block), 8-way data/sequence parallel.

Sharding: cores 0-3 take batch 0, cores 4-7 take batch 1. Within a batch each
core owns 392 query tokens (= 2 frames). K/V for the whole batch are computed
on every core (duplicated, cheap); everything else is sharded by query token.

Per-core inputs are frame-PERMUTED on the host so the same SPMD program works
on every core: the core's own 2 query frames are always frames 0,1 of its
input. All cross-frame ops (stage-2 softmax over frames, output sum over
frames) are frame-order invariant, and the cls token attends to all tokens
(order invariant), so the permutation changes nothing except which rows the
core's outputs correspond to.

Layout strategy: activations are kept FEATURE-major on chip ([C on partitions,
tokens on free]) so chained matmuls need no transposes; LN is done token-major
(free-axis reductions) with PE transposes between. Stage-1 attention computes
scores TRANSPOSED ([keys, queries]) so the softmax denominator comes from a
ones-matmul and the attn@V product needs no transpose of the attention matrix.

Note: the reference computes kv2 = x1 @ Wkv but only uses the k half (v2 is
dead code), so only Wkv[:, :C] is used here.
"""

import sys

import numpy as np

if "/opt/trn_rl_repo" not in sys.path:
    sys.path.insert(0, "/opt/trn_rl_repo")

import concourse.bass as bass
from concourse import bacc
import concourse.mybir as mybir
import concourse.tile as tile
from contextlib import ExitStack

F32 = mybir.dt.float32
F32R = mybir.dt.float32r
AF = mybir.ActivationFunctionType
ALU = mybir.AluOpType

C = 768
CO = 6          # C / 128 contraction chunks
H = 12
NPAIR = 6       # head pairs
D = 64
F = 8           # frames
P = 196         # patches per frame
S = F * P       # 1568 non-cls tokens
N = 1 + S       # 1569
Q = 392         # queries per core (2 frames)
NQ = 1 + Q      # cls + queries
NQP = NQ + 1    # NQ padded even (fp32r matmul dst needs even free count)
NP2 = N + 1     # N padded even
HID = 4 * C
HO = 24         # HID / 128
SCALE = 0.125   # d ** -0.5
EPS = 1e-5

# token chunks used for token-major V and for partition-chunked attention keys:
# per frame two chunks (128 + 68 patches), cls token as final chunk idx 16
VCHUNKS = []
for _f in range(F):
    VCHUNKS.append((1 + P * _f, 128))
    VCHUNKS.append((1 + P * _f + 128, 68))
VCHUNKS.append((0, 1))
NCH = len(VCHUNKS)  # 17

# token tiles for K-producing matmuls (free dim <= 512)
KTOKS = [(0, 512), (512, 512), (1024, 512), (1536, NP2 - 1536)]
# token-major tiles covering the NQ output rows
TT4 = [(0, 128), (128, 128), (256, 128), (384, NQ - 384)]
# token-major tiles covering all N rows (for LN1)
TTN = [(t * 128, min(128, N - t * 128)) for t in range((N + 127) // 128)]


def _bcast_ap(vec_ap, parts=128):
    """DMA access pattern replicating a 1-D dram vector across partitions."""
    return bass.AP(
        tensor=vec_ap.tensor,
        offset=vec_ap.offset,
        ap=[[0, parts]] + [list(a) for a in vec_ap.ap],
    )


def _vseg(vsb, ch, cn, jl, hl):
    """128-wide contiguous [v | 0] / [0 | v] stationary operand for head
    (pair jl, half hl): vsb stores per pair [v_even(64) | zeros(64) |
    v_odd(64)] so a 64-row result lands in either psum half without column
    tiling (which fp32r rejects)."""
    base = 192 * jl
    if hl == 0:
        return vsb[:cn, ch, base : base + 128]
    return vsb[:cn, ch, base + 64 : base + 192]


def _oseg(onez, cn, hl):
    """Same trick for the softmax-denominator ones vector."""
    if hl == 0:
        return onez[:cn, 64:192]
    return onez[:cn, 0:128]


def _mm(nc, out, lhsT, rhs, first=True, last=True, tile_position=None):
    nc.tensor.matmul(
        out,
        lhsT.bitcast(F32R),
        rhs.bitcast(F32R),
        start=first,
        stop=last,
        tile_position=tile_position,
    )


def _layernorm_tile(nc, pools, xt, pt, eps_t, out_t):
    """Token-major LN of xt[:pt, :C] into out_t[:pt, :C]. The per-channel
    gamma/beta are NOT applied here — they are folded into the feature-major
    transpose evacuation (where channel lives on partitions)."""
    stats = pools.tile([128, 3, 6], F32, tag="ln_stats")
    xv = xt[:pt].rearrange("p (s q) -> p s q", s=3)
    for sg in range(3):
        nc.vector.bn_stats(out=stats[:pt, sg], in_=xv[:, sg])
    mv = pools.tile([128, 2], F32, tag="ln_mv")
    nc.vector.bn_aggr(out=mv[:pt], in_=stats[:pt])
    rs = pools.tile([128, 1], F32, tag="ln_rs")
    nc.scalar.activation(
        out=rs[:pt], in_=mv[:pt, 1:2], func=AF.Sqrt, bias=eps_t[:pt], scale=1.0
    )
    nc.vector.reciprocal(out=rs[:pt], in_=rs[:pt])
    nmr = pools.tile([128, 1], F32, tag="ln_nmr")
    nc.vector.tensor_scalar(
        nmr[:pt], mv[:pt, 0:1], rs[:pt], -1.0, ALU.mult, ALU.mult
    )
    nc.scalar.activation(
        out=out_t[:pt],
        in_=xt[:pt],
        func=AF.Identity,
        bias=nmr[:pt],
        scale=rs[:pt],
    )


def build_nc():
    nc = bacc.Bacc("TRN2", target_bir_lowering=False)

    xp_d = nc.dram_tensor("xp", (N, C), F32, kind="ExternalInput")
    wqkv_d = nc.dram_tensor("wqkv", (C, 3 * C), F32, kind="ExternalInput")
    wq_d = nc.dram_tensor("wq", (C, C), F32, kind="ExternalInput")
    wk2_d = nc.dram_tensor("wk2", (C, C), F32, kind="ExternalInput")
    wp_d = nc.dram_tensor("wp", (C, C), F32, kind="ExternalInput")
    w1_d = nc.dram_tensor("w1", (C, HID), F32, kind="ExternalInput")
    w2_d = nc.dram_tensor("w2", (HID, C), F32, kind="ExternalInput")
    g1_d = nc.dram_tensor("g1", (C,), F32, kind="ExternalInput")
    b1_d = nc.dram_tensor("b1", (C,), F32, kind="ExternalInput")
    g2_d = nc.dram_tensor("g2", (C,), F32, kind="ExternalInput")
    b2_d = nc.dram_tensor("b2", (C,), F32, kind="ExternalInput")
    bp_d = nc.dram_tensor("bp", (C,), F32, kind="ExternalInput")
    bf1_d = nc.dram_tensor("bf1", (HID,), F32, kind="ExternalInput")
    bf2_d = nc.dram_tensor("bf2", (C,), F32, kind="ExternalInput")
    ident_d = nc.dram_tensor("ident", (128, 128), F32, kind="ExternalInput")
    onez_d = nc.dram_tensor("onez", (128, 192), F32, kind="ExternalInput")
    e2_d = nc.dram_tensor("e2", (128, 128), F32, kind="ExternalInput")
    out_d = nc.dram_tensor("out", (NQ, C), F32, kind="ExternalOutput")

    wqkv = wqkv_d[:].rearrange("(o p) n -> p o n", p=128)
    wq_fm = wq_d[:].rearrange("(o p) n -> p o n", p=128)
    wk2_fm = wk2_d[:].rearrange("(o p) n -> p o n", p=128)
    wp_fm = wp_d[:].rearrange("(o p) n -> p o n", p=128)
    w1_fm = w1_d[:].rearrange("(o p) n -> p o n", p=128)
    w2_fm = w2_d[:].rearrange("(m p) n -> p m n", p=128)

    with tile.TileContext(nc) as tc, ExitStack() as root:
        consts = root.enter_context(tc.tile_pool(name="consts", bufs=1))
        outp = root.enter_context(tc.tile_pool(name="outp", bufs=1))

        ident = consts.tile([128, 128], F32)
        nc.gpsimd.dma_start(out=ident, in_=ident_d[:])
        onez = consts.tile([128, 192], F32R)
        nc.gpsimd.dma_start(out=onez, in_=onez_d[:].bitcast(F32R))
        e2 = consts.tile([128, 128], F32R)
        nc.gpsimd.dma_start(out=e2, in_=e2_d[:].bitcast(F32R))
        bp_sb = consts.tile([128, CO], F32)
        nc.gpsimd.dma_start(out=bp_sb, in_=bp_d[:].rearrange("(o p) -> p o", p=128))
        bf1_sb = consts.tile([128, HO], F32)
        nc.gpsimd.dma_start(out=bf1_sb, in_=bf1_d[:].rearrange("(o p) -> p o", p=128))
        bf2_sb = consts.tile([128, CO], F32)
        nc.gpsimd.dma_start(out=bf2_sb, in_=bf2_d[:].rearrange("(o p) -> p o", p=128))
        eps_t = consts.tile([128, 1], F32)
        nc.vector.memset(eps_t, EPS)

        # attention block output (feature-major): col 0 = cls, cols 1.. = queries
        outT = outp.tile([128, NPAIR, NQP], F32R)

        with ExitStack() as s_attn:
            xnTp = s_attn.enter_context(tc.tile_pool(name="xnTp", bufs=1))
            xnT = xnTp.tile([128, CO, NP2], F32R)

            # ---------- Phase A1: LN1 (token-major) + transpose to xnT ----------
            with ExitStack() as ph:
                xio = ph.enter_context(tc.tile_pool(name="xio", bufs=3))
                lnp = ph.enter_context(tc.tile_pool(name="lnp", bufs=4))
                gbp = ph.enter_context(tc.tile_pool(name="gbp", bufs=1))
                tps = ph.enter_context(tc.tile_pool(name="tps", bufs=4, space="PSUM"))
                g1f = gbp.tile([128, CO], F32, tag="g1f")
                nc.sync.dma_start(out=g1f, in_=g1_d[:].rearrange("(o p) -> p o", p=128))
                b1f = gbp.tile([128, CO], F32, tag="b1f")
                nc.sync.dma_start(out=b1f, in_=b1_d[:].rearrange("(o p) -> p o", p=128))
                for t0, pt in TTN:
                    xt = xio.tile([128, C], F32, tag="xt")
                    nc.sync.dma_start(out=xt[:pt], in_=xp_d[t0 : t0 + pt])
                    xn = xio.tile([128, C], F32, tag="xn")
                    _layernorm_tile(nc, lnp, xt, pt, eps_t, xn)
                    for o in range(CO):
                        pst = tps.tile([128, 128], F32, tag="tp")
                        nc.tensor.transpose(
                            pst[:, :pt],
                            xn[:pt, o * 128 : (o + 1) * 128],
                            ident[:pt, :pt],
                        )
                        # fused gamma/beta: channel is the partition dim here
                        nc.vector.tensor_scalar(
                            xnT[:, o, t0 : t0 + pt],
                            pst[:, :pt],
                            g1f[:, o : o + 1],
                            b1f[:, o : o + 1],
                            ALU.mult,
                            ALU.add,
                        )

            # ---------- head-group loop: QKV + stage-1 attention + cls ----------
            with ExitStack() as s_x1:
                x1Tp = s_x1.enter_context(tc.tile_pool(name="x1Tp", bufs=1))
                x1T = x1Tp.tile([128, NPAIR, F, Q], F32R)

                for g in range(2):
                    pairs = [3 * g, 3 * g + 1, 3 * g + 2]
                    with ExitStack() as sg:
                        kqvp = sg.enter_context(tc.tile_pool(name="kqv", bufs=1))
                        kT = kqvp.tile([128, 3, NP2], F32R, tag="kT")
                        qT = kqvp.tile([128, 3, NQP], F32R, tag="qT")
                        vsb = kqvp.tile([128, NCH, 576], F32R, tag="vsb")
                        for _ch in range(NCH):
                            for _jl in range(3):
                                nc.vector.memset(
                                    vsb[
                                        :, _ch, 192 * _jl + 64 : 192 * _jl + 128
                                    ].bitcast(F32),
                                    0.0,
                                )

                        # ----- A-g: K/Q (feature-major) and V (token-major) -----
                        with ExitStack() as sa:
                            wkq = sa.enter_context(tc.tile_pool(name="wkq", bufs=2))
                            wvv = sa.enter_context(tc.tile_pool(name="wvv", bufs=1))
                            kqps = sa.enter_context(
                                tc.tile_pool(name="kqps", bufs=2, space="PSUM")
                            )
                            vps = sa.enter_context(
                                tc.tile_pool(name="vps", bufs=4, space="PSUM")
                            )
                            for jl, j in enumerate(pairs):
                                wk_t = wkq.tile([128, CO, 128], F32R, tag="wk")
                                nc.sync.dma_start(
                                    out=wk_t,
                                    in_=wqkv[:, :, C + j * 128 : C + (j + 1) * 128].bitcast(F32R),
                                )
                                for tt0, tn in KTOKS:
                                    ps = kqps.tile([128, 512], F32, tag="kqps")
                                    for o in range(CO):
                                        _mm(
                                            nc,
                                            ps[:, :tn],
                                            wk_t[:, o],
                                            xnT[:, o, tt0 : tt0 + tn],
                                            o == 0,
                                            o == CO - 1,
                                        )
                                    nc.any.tensor_copy(
                                        out=kT[:, jl, tt0 : tt0 + tn], in_=ps[:, :tn]
                                    )
                                wq_t = wkq.tile([128, CO, 128], F32R, tag="wqg")
                                nc.sync.dma_start(
                                    out=wq_t, in_=wqkv[:, :, j * 128 : (j + 1) * 128].bitcast(F32R)
                                )
                                ps = kqps.tile([128, 512], F32, tag="kqps")
                                for o in range(CO):
                                    _mm(
                                        nc,
                                        ps[:, :NQP],
                                        wq_t[:, o],
                                        xnT[:, o, 0:NQP],
                                        o == 0,
                                        o == CO - 1,
                                    )
                                nc.any.tensor_copy(out=qT[:, jl], in_=ps[:, :NQP])
                            # V for this head group (6 heads = 384 cols)
                            wv_t = wvv.tile([128, CO, 384], F32R, tag="wv")
                            nc.sync.dma_start(
                                out=wv_t,
                                in_=wqkv[:, :, 2 * C + g * 384 : 2 * C + (g + 1) * 384].bitcast(F32R),
                            )
                            for ch, (c0, cn) in enumerate(VCHUNKS):
                                ps = vps.tile([128, 384], F32, tag="vps")
                                for o in range(CO):
                                    _mm(
                                        nc,
                                        ps[:cn],
                                        xnT[:, o, c0 : c0 + cn],
                                        wv_t[:, o],
                                        o == 0,
                                        o == CO - 1,
                                    )
                                vv = vsb[:cn, ch].rearrange("p (j s) -> p j s", s=192)
                                pv = ps[:cn].rearrange("p (j h e) -> p j h e", j=3, h=2)
                                nc.any.tensor_copy(out=vv[:, :, 0:64], in_=pv[:, :, 0])
                                nc.any.tensor_copy(
                                    out=vv[:, :, 128:192], in_=pv[:, :, 1]
                                )

                        # ----- B-g: stage-1 trajectory attention -----
                        with ExitStack() as sb:
                            scps = sb.enter_context(
                                tc.tile_pool(name="scps", bufs=2, space="PSUM")
                            )
                            sxps = sb.enter_context(
                                tc.tile_pool(name="sxps", bufs=2, space="PSUM")
                            )
                            ssps = sb.enter_context(
                                tc.tile_pool(name="ssps", bufs=2, space="PSUM")
                            )
                            expp = sb.enter_context(tc.tile_pool(name="expp", bufs=5))
                            rcpp = sb.enter_context(tc.tile_pool(name="rcpp", bufs=2))
                            for jl, j in enumerate(pairs):
                                for fg in range(4):  # 2 frames per group
                                    ext = []
                                    for hl in range(2):
                                        dlo = hl * 64
                                        exh = []
                                        for fi in range(2):
                                            f = fg * 2 + fi
                                            sps = scps.tile(
                                                [128, 2, 512], F32, tag="sc"
                                            )
                                            for ci in range(2):
                                                c0, cn = VCHUNKS[2 * f + ci]
                                                _mm(
                                                    nc,
                                                    sps[:cn, ci, :Q],
                                                    kT[dlo : dlo + 64, jl, c0 : c0 + cn],
                                                    qT[dlo : dlo + 64, jl, 1:NQ],
                                                )
                                            ex = expp.tile([128, 2, Q], F32R, tag="ex")
                                            nc.scalar.activation(
                                                out=ex,
                                                in_=sps[:, :, :Q],
                                                func=AF.Exp,
                                                scale=SCALE,
                                            )
                                            exh.append(ex)
                                        ext.append(exh)
                                    for fi in range(2):
                                        f = fg * 2 + fi
                                        sx = sxps.tile([128, Q], F32, tag="sx")
                                        ss = ssps.tile([128, Q], F32, tag="ss")
                                        for hl in range(2):
                                            gh = jl * 2 + hl
                                            for ci in range(2):
                                                c0, cn = VCHUNKS[2 * f + ci]
                                                exc = ext[hl][fi][:cn, ci]
                                                first = hl == 0 and ci == 0
                                                last = hl == 1 and ci == 1
                                                _mm(
                                                    nc,
                                                    sx,
                                                    _vseg(vsb, 2 * f + ci, cn, jl, hl),
                                                    exc,
                                                    first,
                                                    last,
                                                )
                                                _mm(
                                                    nc,
                                                    ss,
                                                    _oseg(onez, cn, hl),
                                                    exc,
                                                    first,
                                                    last,
                                                )
                                        rc = rcpp.tile([128, Q], F32, tag="rc")
                                        nc.vector.reciprocal(out=rc, in_=ss)
                                        nc.vector.tensor_tensor(
                                            out=x1T[:, j, f],
                                            in0=sx,
                                            in1=rc,
                                            op=ALU.mult,
                                        )

                        # ----- B5-g: cls attention (over all tokens) -----
                        with ExitStack() as s5:
                            cps = s5.enter_context(
                                tc.tile_pool(name="cps", bufs=2, space="PSUM")
                            )
                            czps = s5.enter_context(
                                tc.tile_pool(name="czps", bufs=2, space="PSUM")
                            )
                            cops = s5.enter_context(
                                tc.tile_pool(name="cops", bufs=2, space="PSUM")
                            )
                            cexp = s5.enter_context(tc.tile_pool(name="cexp", bufs=2))
                            czrp = s5.enter_context(tc.tile_pool(name="czrp", bufs=2))
                            for jl, j in enumerate(pairs):
                                # column 0 = cls query; column 1 = padding (the
                                # first real query) so fp32r dst free counts
                                # stay even
                                co_ps = cops.tile([128, 2], F32, tag="co")
                                zps = czps.tile([128, NCH, 2], F32, tag="cz")
                                zrr = czrp.tile([128, 1], F32, tag="zr")
                                exs = []
                                for hl in range(2):
                                    dlo = hl * 64
                                    sps = cps.tile([128, NCH, 2], F32, tag="cs")
                                    for ch, (c0, cn) in enumerate(VCHUNKS):
                                        _mm(
                                            nc,
                                            sps[:cn, ch],
                                            kT[dlo : dlo + 64, jl, c0 : c0 + cn],
                                            qT[dlo : dlo + 64, jl, 0:2],
                                        )
                                    ex = cexp.tile([128, NCH, 2], F32R, tag="cex")
                                    nc.scalar.activation(
                                        out=ex, in_=sps, func=AF.Exp, scale=SCALE
                                    )
                                    exs.append(ex)
                                # per-chunk partition-restricted sums so junk
                                # lanes (partitions past chunk size) are never read
                                for ch, (c0, cn) in enumerate(VCHUNKS):
                                    for hl in range(2):
                                        _mm(
                                            nc,
                                            zps[:, ch],
                                            _oseg(onez, cn, hl),
                                            exs[hl][:cn, ch],
                                            hl == 0,
                                            hl == 1,
                                        )
                                nc.vector.reduce_sum(
                                    out=zrr,
                                    in_=zps[:, :, 0],
                                    axis=mybir.AxisListType.X,
                                )
                                nc.vector.reciprocal(out=zrr, in_=zrr)
                                for hl in range(2):
                                    gh = jl * 2 + hl
                                    for ch, (c0, cn) in enumerate(VCHUNKS):
                                        _mm(
                                            nc,
                                            co_ps,
                                            _vseg(vsb, ch, cn, jl, hl),
                                            exs[hl][:cn, ch],
                                            hl == 0 and ch == 0,
                                            hl == 1 and ch == NCH - 1,
                                        )
                                nc.vector.tensor_scalar_mul(
                                    outT[:, j, 0:1], co_ps[:, 0:1], zrr
                                )

                # ---------- Phase C: stage-2 frame attention ----------
                with ExitStack() as sC:
                    xdp = sC.enter_context(tc.tile_pool(name="xdp", bufs=1))
                    q2p = sC.enter_context(tc.tile_pool(name="q2p", bufs=1))
                    wsC = sC.enter_context(tc.tile_pool(name="wsC", bufs=2))
                    k2ps = sC.enter_context(
                        tc.tile_pool(name="k2ps", bufs=3, space="PSUM")
                    )
                    lps = sC.enter_context(tc.tile_pool(name="lps", bufs=2, space="PSUM"))
                    prp = sC.enter_context(tc.tile_pool(name="prp", bufs=3))
                    elp = sC.enter_context(tc.tile_pool(name="elp", bufs=2))
                    zzp = sC.enter_context(tc.tile_pool(name="zzp", bufs=2))

                    # x_diag: query i<196 -> frame 0, else frame 1 (frame-permuted)
                    xdT = xdp.tile([128, CO, Q], F32R)
                    nc.any.tensor_copy(out=xdT[:, :, 0:P], in_=x1T[:, :, 0, 0:P])
                    nc.any.tensor_copy(out=xdT[:, :, P:Q], in_=x1T[:, :, 1, P:Q])

                    q2T = q2p.tile([128, CO, Q], F32)
                    for j in range(NPAIR):
                        wq_t = wsC.tile([128, CO, 128], F32R, tag="wqC")
                        nc.sync.dma_start(
                            out=wq_t, in_=wq_fm[:, :, j * 128 : (j + 1) * 128].bitcast(F32R)
                        )
                        ps = k2ps.tile([128, Q], F32, tag="k2")
                        for o in range(CO):
                            _mm(nc, ps, wq_t[:, o], xdT[:, o], o == 0, o == CO - 1)
                        nc.any.tensor_copy(out=q2T[:, j], in_=ps)

                    for j in range(NPAIR):
                        wk_t = wsC.tile([128, CO, 128], F32R, tag="wkC")
                        nc.sync.dma_start(
                            out=wk_t, in_=wk2_fm[:, :, j * 128 : (j + 1) * 128].bitcast(F32R)
                        )
                        el = elp.tile([128, F, Q], F32, tag="el")
                        for f in range(F):
                            ps = k2ps.tile([128, Q], F32, tag="k2")
                            for o in range(CO):
                                _mm(
                                    nc,
                                    ps,
                                    wk_t[:, o],
                                    x1T[:, o, f],
                                    o == 0,
                                    o == CO - 1,
                                )
                            pr = prp.tile([128, Q], F32R, tag="pr")
                            nc.vector.tensor_tensor(
                                out=pr, in0=ps, in1=q2T[:, j], op=ALU.mult
                            )
                            lp = lps.tile([128, Q], F32, tag="lg")
                            _mm(nc, lp, e2, pr)
                            nc.scalar.activation(
                                out=el[:, f], in_=lp, func=AF.Exp, scale=SCALE
                            )
                        # Z = sum_f el[f]: tree split across gpsimd + DVE
                        zz = zzp.tile([128, Q], F32, tag="zz")
                        za = zzp.tile([128, Q], F32, tag="za")
                        zb = zzp.tile([128, Q], F32, tag="zb")
                        nc.gpsimd.tensor_tensor(
                            out=za, in0=el[:, 0], in1=el[:, 1], op=ALU.add
                        )
                        nc.vector.tensor_tensor(
                            out=zb, in0=el[:, 2], in1=el[:, 3], op=ALU.add
                        )
                        nc.gpsimd.tensor_tensor(
                            out=za, in0=za, in1=el[:, 4], op=ALU.add
                        )
                        nc.vector.tensor_tensor(
                            out=zb, in0=zb, in1=el[:, 5], op=ALU.add
                        )
                        nc.gpsimd.tensor_tensor(
                            out=za, in0=za, in1=el[:, 6], op=ALU.add
                        )
                        nc.vector.tensor_tensor(
                            out=zb, in0=zb, in1=el[:, 7], op=ALU.add
                        )
                        nc.vector.tensor_tensor(out=zz, in0=za, in1=zb, op=ALU.add)
                        nc.vector.reciprocal(out=zz, in_=zz)
                        acc = outT[:, j, 1:NQ]
                        nc.vector.tensor_tensor(
                            out=acc, in0=x1T[:, j, 0], in1=el[:, 0], op=ALU.mult
                        )
                        for f in range(1, F):
                            tm = prp.tile([128, Q], F32, tag="tm")
                            # odd-f multiplies ride the otherwise-idle gpsimd
                            eng = nc.gpsimd if f % 2 else nc.vector
                            eng.tensor_tensor(
                                out=tm, in0=x1T[:, j, f], in1=el[:, f], op=ALU.mult
                            )
                            nc.vector.tensor_tensor(
                                out=acc, in0=acc, in1=tm, op=ALU.add
                            )
                        nc.vector.tensor_tensor(out=acc, in0=acc, in1=zz, op=ALU.mult)

        # ---------- Phase D: proj + residual + LN2 + MLP + output ----------
        with ExitStack() as sD:
            wpp = sD.enter_context(tc.tile_pool(name="wpD", bufs=4))
            w2p = sD.enter_context(tc.tile_pool(name="w2D", bufs=3))
            xop = sD.enter_context(tc.tile_pool(name="xoD", bufs=1))
            pjp = sD.enter_context(tc.tile_pool(name="pjD", bufs=1))
            h1p = sD.enter_context(tc.tile_pool(name="h1D", bufs=1))
            m2p = sD.enter_context(tc.tile_pool(name="m2D", bufs=1))
            xn2p = sD.enter_context(tc.tile_pool(name="xn2D", bufs=1))
            gbD = sD.enter_context(tc.tile_pool(name="gbD", bufs=1))
            finp = sD.enter_context(tc.tile_pool(name="finp", bufs=2))
            ln2p = sD.enter_context(tc.tile_pool(name="ln2p", bufs=4))
            mps = sD.enter_context(tc.tile_pool(name="mps", bufs=2, space="PSUM"))
            tps2 = sD.enter_context(tc.tile_pool(name="tps2", bufs=4, space="PSUM"))

            g2f = gbD.tile([128, CO], F32, tag="g2f")
            nc.sync.dma_start(out=g2f, in_=g2_d[:].rearrange("(o p) -> p o", p=128))
            b2f = gbD.tile([128, CO], F32, tag="b2f")
            nc.sync.dma_start(out=b2f, in_=b2_d[:].rearrange("(o p) -> p o", p=128))

            xown = xop.tile([128, 4, C], F32, tag="xo")
            for tt, (t0, pt) in enumerate(TT4):
                nc.sync.dma_start(out=xown[:pt, tt], in_=xp_d[t0 : t0 + pt])

            projT = pjp.tile([128, CO, NQ], F32)
            for j in range(CO):
                wp_t = wpp.tile([128, CO, 128], F32R, tag="wpD")
                nc.sync.dma_start(out=wp_t, in_=wp_fm[:, :, j * 128 : (j + 1) * 128].bitcast(F32R))
                ps = mps.tile([128, 512], F32, tag="mm")
                for o in range(CO):
                    _mm(nc, ps[:, :NQP], wp_t[:, o], outT[:, o], o == 0, o == CO - 1)
                nc.vector.tensor_scalar(
                    projT[:, j], ps[:, :NQ], bp_sb[:, j : j + 1], None, ALU.add
                )

            # residual + LN2 (token-major), then transpose to xn2T
            x2 = xop.tile([128, 4, C], F32, tag="x2")
            xn2T = xn2p.tile([128, CO, NQP], F32R)
            for tt, (t0, pt) in enumerate(TT4):
                for o in range(CO):
                    pst = tps2.tile([128, 128], F32, tag="tp2")
                    nc.tensor.transpose(
                        pst[:pt], projT[:, o, t0 : t0 + pt], ident
                    )
                    nc.vector.tensor_tensor(
                        out=x2[:pt, tt, o * 128 : (o + 1) * 128],
                        in0=pst[:pt],
                        in1=xown[:pt, tt, o * 128 : (o + 1) * 128],
                        op=ALU.add,
                    )
                xn2 = finp.tile([128, C], F32, tag="xn2")
                _layernorm_tile(nc, ln2p, x2[:, tt], pt, eps_t, xn2)
                for o in range(CO):
                    pst = tps2.tile([128, 128], F32, tag="tp2")
                    nc.tensor.transpose(
                        pst[:, :pt], xn2[:pt, o * 128 : (o + 1) * 128], ident[:pt, :pt]
                    )
                    nc.vector.tensor_scalar(
                        xn2T[:, o, t0 : t0 + pt],
                        pst[:, :pt],
                        g2f[:, o : o + 1],
                        b2f[:, o : o + 1],
                        ALU.mult,
                        ALU.add,
                    )

            # MLP up + gelu
            h1T = h1p.tile([128, HO, NQP], F32R)
            for m in range(HO):
                w1_t = wpp.tile([128, CO, 128], F32R, tag="w1D")
                nc.sync.dma_start(out=w1_t, in_=w1_fm[:, :, m * 128 : (m + 1) * 128].bitcast(F32R))
                ps = mps.tile([128, 512], F32, tag="mm")
                for o in range(CO):
                    _mm(nc, ps[:, :NQP], w1_t[:, o], xn2T[:, o], o == 0, o == CO - 1)
                nc.scalar.activation(
                    out=h1T[:, m, :NQ],
                    in_=ps[:, :NQ],
                    func=AF.Gelu,
                    bias=bf1_sb[:, m : m + 1],
                    scale=1.0,
                )

            # MLP down
            m2T = m2p.tile([128, CO, NQ], F32)
            for j in range(CO):
                w2_t = w2p.tile([128, HO, 128], F32R, tag="w2D")
                nc.sync.dma_start(out=w2_t, in_=w2_fm[:, :, j * 128 : (j + 1) * 128].bitcast(F32R))
                ps = mps.tile([128, 512], F32, tag="mm")
                for m in range(HO):
                    _mm(nc, ps[:, :NQP], w2_t[:, m], h1T[:, m], m == 0, m == HO - 1)
                nc.vector.tensor_scalar(
                    m2T[:, j], ps[:, :NQ], bf2_sb[:, j : j + 1], None, ALU.add
                )

            # transpose back to token-major, final residual, store
            for tt, (t0, pt) in enumerate(TT4):
                fo = finp.tile([128, C], F32, tag="fo")
                for o in range(CO):
                    pst = tps2.tile([128, 128], F32, tag="tp2")
                    nc.tensor.transpose(pst[:pt], m2T[:, o, t0 : t0 + pt], ident)
                    nc.vector.tensor_tensor(
                        out=fo[:pt, o * 128 : (o + 1) * 128],
                        in0=pst[:pt],
                        in1=x2[:pt, tt, o * 128 : (o + 1) * 128],
                        op=ALU.add,
                    )
                nc.sync.dma_start(out=out_d[t0 : t0 + pt], in_=fo[:pt])

    nc.compile()
    return nc


_CACHE = {}


def _get_nc():
    if "nc" not in _CACHE:
        _CACHE["nc"] = build_nc()
    return _CACHE["nc"]


def _make_in_maps(inputs):
    f32 = np.float32
    x = np.asarray(inputs["x"], f32)
    wqkv = np.ascontiguousarray(np.asarray(inputs["Wqkv"], f32))
    wq = np.ascontiguousarray(np.asarray(inputs["Wq"], f32))
    wk2 = np.ascontiguousarray(np.asarray(inputs["Wkv"], f32)[:, :C])
    wp = np.ascontiguousarray(np.asarray(inputs["Wp"], f32))
    w1 = np.ascontiguousarray(np.asarray(inputs["W1"], f32))
    w2 = np.ascontiguousarray(np.asarray(inputs["W2"], f32))
    ident = np.eye(128, dtype=f32)
    onez = np.zeros((128, 192), dtype=f32)
    onez[:, 64:128] = 1.0
    e2 = np.zeros((128, 128), dtype=f32)
    e2[:64, :64] = 1.0
    e2[64:, 64:] = 1.0
    common = dict(
        wqkv=wqkv,
        wq=wq,
        wk2=wk2,
        wp=wp,
        w1=w1,
        w2=w2,
        g1=np.asarray(inputs["g1"], f32),
        b1=np.asarray(inputs["b1"], f32),
        g2=np.asarray(inputs["g2"], f32),
        b2=np.asarray(inputs["b2"], f32),
        bp=np.asarray(inputs["bp"], f32),
        bf1=np.asarray(inputs["bf1"], f32),
        bf2=np.asarray(inputs["bf2"], f32),
        ident=ident,
        onez=onez,
        e2=e2,
    )
    in_maps = []
    for c in range(8):
        b, cl = c // 4, c % 4
        f0 = 2 * cl
        order = [(f0 + i) % F for i in range(F)]
        xb = x[b]
        xp = np.concatenate(
            [xb[:1], xb[1:].reshape(F, P, C)[order].reshape(S, C)], axis=0
        )
        m = dict(common)
        m["xp"] = np.ascontiguousarray(xp)
        in_maps.append(m)
    return in_maps


def kernel(**inputs):
    from concourse.bass_utils import run_bass_kernel_spmd

    in_maps = _make_in_maps(inputs)
    res = run_bass_kernel_spmd(_get_nc(), in_maps, core_ids=list(range(8)))
    outs = res.results
    x = np.asarray(inputs["x"])
    full = np.empty((x.shape[0], N, C), dtype=np.float32)
    for c in range(8):
        r = outs[c]["out"]
        b, cl = c // 4, c % 4
        if cl == 0:
            full[b, 0] = r[0]
        full[b, 1 + Q * cl : 1 + Q * (cl + 1)] = r[1:]
    return full


# revision 20
# speedup vs baseline: 3.3947x; 1.0011x over previous
"""Trainium2 Bass/Tile kernel for nn_Block_40475771797636 (trajectory-attention
transformer block), 8-way data/sequence parallel.

Sharding: cores 0-3 take batch 0, cores 4-7 take batch 1. Within a batch each
core owns 392 query tokens (= 2 frames). K/V for the whole batch are computed
on every core (duplicated, cheap); everything else is sharded by query token.

Per-core inputs are frame-PERMUTED on the host so the same SPMD program works
on every core: the core's own 2 query frames are always frames 0,1 of its
input. All cross-frame ops (stage-2 softmax over frames, output sum over
frames) are frame-order invariant, and the cls token attends to all tokens
(order invariant), so the permutation changes nothing except which rows the
core's outputs correspond to.

Layout strategy: activations are kept FEATURE-major on chip ([C on partitions,
tokens on free]) so chained matmuls need no transposes; LN is done token-major
(free-axis reductions) with PE transposes between. Stage-1 attention computes
scores TRANSPOSED ([keys, queries]) so the softmax denominator comes from a
ones-matmul and the attn@V product needs no transpose of the attention matrix.

Note: the reference computes kv2 = x1 @ Wkv but only uses the k half (v2 is
dead code), so only Wkv[:, :C] is used here.
"""

import sys

import numpy as np

if "/opt/trn_rl_repo" not in sys.path:
    sys.path.insert(0, "/opt/trn_rl_repo")

import concourse.bass as bass
from concourse import bacc
import concourse.mybir as mybir
import concourse.tile as tile
from contextlib import ExitStack

F32 = mybir.dt.float32
F32R = mybir.dt.float32r
AF = mybir.ActivationFunctionType
ALU = mybir.AluOpType

C = 768
CO = 6          # C / 128 contraction chunks
H = 12
NPAIR = 6       # head pairs
D = 64
F = 8           # frames
P = 196         # patches per frame
S = F * P       # 1568 non-cls tokens
N = 1 + S       # 1569
Q = 392         # queries per core (2 frames)
NQ = 1 + Q      # cls + queries
NQP = NQ + 1    # NQ padded even (fp32r matmul dst needs even free count)
NP2 = N + 1     # N padded even
HID = 4 * C
HO = 24         # HID / 128
SCALE = 0.125   # d ** -0.5
EPS = 1e-5

# token chunks used for token-major V and for partition-chunked attention keys:
# per frame two chunks (128 + 68 patches), cls token as final chunk idx 16
VCHUNKS = []
for _f in range(F):
    VCHUNKS.append((1 + P * _f, 128))
    VCHUNKS.append((1 + P * _f + 128, 68))
VCHUNKS.append((0, 1))
NCH = len(VCHUNKS)  # 17

# token tiles for K-producing matmuls (free dim <= 512)
KTOKS = [(0, 512), (512, 512), (1024, 512), (1536, NP2 - 1536)]
# token-major tiles covering the NQ output rows
TT4 = [(0, 128), (128, 128), (256, 128), (384, NQ - 384)]
# token-major tiles covering all N rows (for LN1)
TTN = [(t * 128, min(128, N - t * 128)) for t in range((N + 127) // 128)]


def _bcast_ap(vec_ap, parts=128):
    """DMA access pattern replicating a 1-D dram vector across partitions."""
    return bass.AP(
        tensor=vec_ap.tensor,
        offset=vec_ap.offset,
        ap=[[0, parts]] + [list(a) for a in vec_ap.ap],
    )


def _vseg(vsb, ch, cn, jl, hl):
    """128-wide contiguous [v | 0] / [0 | v] stationary operand for head
    (pair jl, half hl): vsb stores per pair [v_even(64) | zeros(64) |
    v_odd(64)] so a 64-row result lands in either psum half without column
    tiling (which fp32r rejects)."""
    base = 192 * jl
    if hl == 0:
        return vsb[:cn, ch, base : base + 128]
    return vsb[:cn, ch, base + 64 : base + 192]


def _oseg(onez, cn, hl):
    """Same trick for the softmax-denominator ones vector."""
    if hl == 0:
        return onez[:cn, 64:192]
    return onez[:cn, 0:128]


def _mm(nc, out, lhsT, rhs, first=True, last=True, tile_position=None):
    nc.tensor.matmul(
        out,
        lhsT.bitcast(F32R),
        rhs.bitcast(F32R),
        start=first,
        stop=last,
        tile_position=tile_position,
    )


def _layernorm_tile(nc, pools, xt, pt, eps_t, out_t):
    """Token-major LN of xt[:pt, :C] into out_t[:pt, :C]. The per-channel
    gamma/beta are NOT applied here — they are folded into the feature-major
    transpose evacuation (where channel lives on partitions)."""
    stats = pools.tile([128, 3, 6], F32, tag="ln_stats")
    xv = xt[:pt].rearrange("p (s q) -> p s q", s=3)
    for sg in range(3):
        nc.vector.bn_stats(out=stats[:pt, sg], in_=xv[:, sg])
    mv = pools.tile([128, 2], F32, tag="ln_mv")
    nc.vector.bn_aggr(out=mv[:pt], in_=stats[:pt])
    rs = pools.tile([128, 1], F32, tag="ln_rs")
    nc.scalar.activation(
        out=rs[:pt], in_=mv[:pt, 1:2], func=AF.Sqrt, bias=eps_t[:pt], scale=1.0
    )
    nc.vector.reciprocal(out=rs[:pt], in_=rs[:pt])
    nmr = pools.tile([128, 1], F32, tag="ln_nmr")
    nc.vector.tensor_scalar(
        nmr[:pt], mv[:pt, 0:1], rs[:pt], -1.0, ALU.mult, ALU.mult
    )
    nc.scalar.activation(
        out=out_t[:pt],
        in_=xt[:pt],
        func=AF.Identity,
        bias=nmr[:pt],
        scale=rs[:pt],
    )


def build_nc():
    nc = bacc.Bacc("TRN2", target_bir_lowering=False)

    xp_d = nc.dram_tensor("xp", (N, C), F32, kind="ExternalInput")
    wqkv_d = nc.dram_tensor("wqkv", (C, 3 * C), F32, kind="ExternalInput")
    wq_d = nc.dram_tensor("wq", (C, C), F32, kind="ExternalInput")
    wk2_d = nc.dram_tensor("wk2", (C, C), F32, kind="ExternalInput")
    wp_d = nc.dram_tensor("wp", (C, C), F32, kind="ExternalInput")
    w1_d = nc.dram_tensor("w1", (C, HID), F32, kind="ExternalInput")
    w2_d = nc.dram_tensor("w2", (HID, C), F32, kind="ExternalInput")
    g1_d = nc.dram_tensor("g1", (C,), F32, kind="ExternalInput")
    b1_d = nc.dram_tensor("b1", (C,), F32, kind="ExternalInput")
    g2_d = nc.dram_tensor("g2", (C,), F32, kind="ExternalInput")
    b2_d = nc.dram_tensor("b2", (C,), F32, kind="ExternalInput")
    bp_d = nc.dram_tensor("bp", (C,), F32, kind="ExternalInput")
    bf1_d = nc.dram_tensor("bf1", (HID,), F32, kind="ExternalInput")
    bf2_d = nc.dram_tensor("bf2", (C,), F32, kind="ExternalInput")
    ident_d = nc.dram_tensor("ident", (128, 128), F32, kind="ExternalInput")
    onez_d = nc.dram_tensor("onez", (128, 192), F32, kind="ExternalInput")
    e2_d = nc.dram_tensor("e2", (128, 128), F32, kind="ExternalInput")
    out_d = nc.dram_tensor("out", (NQ, C), F32, kind="ExternalOutput")

    wqkv = wqkv_d[:].rearrange("(o p) n -> p o n", p=128)
    wq_fm = wq_d[:].rearrange("(o p) n -> p o n", p=128)
    wk2_fm = wk2_d[:].rearrange("(o p) n -> p o n", p=128)
    wp_fm = wp_d[:].rearrange("(o p) n -> p o n", p=128)
    w1_fm = w1_d[:].rearrange("(o p) n -> p o n", p=128)
    w2_fm = w2_d[:].rearrange("(m p) n -> p m n", p=128)

    with tile.TileContext(nc) as tc, ExitStack() as root:
        consts = root.enter_context(tc.tile_pool(name="consts", bufs=1))
        outp = root.enter_context(tc.tile_pool(name="outp", bufs=1))

        ident = consts.tile([128, 128], F32)
        nc.gpsimd.dma_start(out=ident, in_=ident_d[:])
        onez = consts.tile([128, 192], F32R)
        nc.gpsimd.dma_start(out=onez, in_=onez_d[:].bitcast(F32R))
        e2 = consts.tile([128, 128], F32R)
        nc.gpsimd.dma_start(out=e2, in_=e2_d[:].bitcast(F32R))
        bp_sb = consts.tile([128, CO], F32)
        nc.gpsimd.dma_start(out=bp_sb, in_=bp_d[:].rearrange("(o p) -> p o", p=128))
        bf1_sb = consts.tile([128, HO], F32)
        nc.gpsimd.dma_start(out=bf1_sb, in_=bf1_d[:].rearrange("(o p) -> p o", p=128))
        bf2_sb = consts.tile([128, CO], F32)
        nc.gpsimd.dma_start(out=bf2_sb, in_=bf2_d[:].rearrange("(o p) -> p o", p=128))
        eps_t = consts.tile([128, 1], F32)
        nc.vector.memset(eps_t, EPS)

        # attention block output (feature-major): col 0 = cls, cols 1.. = queries
        outT = outp.tile([128, NPAIR, NQP], F32R)

        with ExitStack() as s_attn:
            xnTp = s_attn.enter_context(tc.tile_pool(name="xnTp", bufs=1))
            xnT = xnTp.tile([128, CO, NP2], F32R)

            # ---------- Phase A1: LN1 (token-major) + transpose to xnT ----------
            with ExitStack() as ph:
                xio = ph.enter_context(tc.tile_pool(name="xio", bufs=5))
                lnp = ph.enter_context(tc.tile_pool(name="lnp", bufs=8))
                gbp = ph.enter_context(tc.tile_pool(name="gbp", bufs=1))
                tps = ph.enter_context(tc.tile_pool(name="tps", bufs=4, space="PSUM"))
                g1f = gbp.tile([128, CO], F32, tag="g1f")
                nc.sync.dma_start(out=g1f, in_=g1_d[:].rearrange("(o p) -> p o", p=128))
                b1f = gbp.tile([128, CO], F32, tag="b1f")
                nc.sync.dma_start(out=b1f, in_=b1_d[:].rearrange("(o p) -> p o", p=128))
                for t0, pt in TTN:
                    xt = xio.tile([128, C], F32, tag="xt")
                    nc.sync.dma_start(out=xt[:pt], in_=xp_d[t0 : t0 + pt])
                    xn = xio.tile([128, C], F32, tag="xn")
                    _layernorm_tile(nc, lnp, xt, pt, eps_t, xn)
                    for o in range(CO):
                        pst = tps.tile([128, 128], F32, tag="tp")
                        nc.tensor.transpose(
                            pst[:, :pt],
                            xn[:pt, o * 128 : (o + 1) * 128],
                            ident[:pt, :pt],
                        )
                        # fused gamma/beta: channel is the partition dim here
                        nc.vector.tensor_scalar(
                            xnT[:, o, t0 : t0 + pt],
                            pst[:, :pt],
                            g1f[:, o : o + 1],
                            b1f[:, o : o + 1],
                            ALU.mult,
                            ALU.add,
                        )

            # ---------- head-group loop: QKV + stage-1 attention + cls ----------
            with ExitStack() as s_x1:
                x1Tp = s_x1.enter_context(tc.tile_pool(name="x1Tp", bufs=1))
                x1T = x1Tp.tile([128, NPAIR, F, Q], F32R)

                for g in range(2):
                    pairs = [3 * g, 3 * g + 1, 3 * g + 2]
                    with ExitStack() as sg:
                        kqvp = sg.enter_context(tc.tile_pool(name="kqv", bufs=1))
                        kT = kqvp.tile([128, 3, NP2], F32R, tag="kT")
                        qT = kqvp.tile([128, 3, NQP], F32R, tag="qT")
                        vsb = kqvp.tile([128, NCH, 576], F32R, tag="vsb")
                        for _ch in range(NCH):
                            for _jl in range(3):
                                nc.vector.memset(
                                    vsb[
                                        :, _ch, 192 * _jl + 64 : 192 * _jl + 128
                                    ].bitcast(F32),
                                    0.0,
                                )

                        # ----- A-g: K/Q (feature-major) and V (token-major) -----
                        with ExitStack() as sa:
                            wkq = sa.enter_context(tc.tile_pool(name="wkq", bufs=2))
                            wvv = sa.enter_context(tc.tile_pool(name="wvv", bufs=1))
                            kqps = sa.enter_context(
                                tc.tile_pool(name="kqps", bufs=2, space="PSUM")
                            )
                            vps = sa.enter_context(
                                tc.tile_pool(name="vps", bufs=4, space="PSUM")
                            )
                            for jl, j in enumerate(pairs):
                                wk_t = wkq.tile([128, CO, 128], F32R, tag="wk")
                                nc.sync.dma_start(
                                    out=wk_t,
                                    in_=wqkv[:, :, C + j * 128 : C + (j + 1) * 128].bitcast(F32R),
                                )
                                for tt0, tn in KTOKS:
                                    ps = kqps.tile([128, 512], F32, tag="kqps")
                                    for o in range(CO):
                                        _mm(
                                            nc,
                                            ps[:, :tn],
                                            wk_t[:, o],
                                            xnT[:, o, tt0 : tt0 + tn],
                                            o == 0,
                                            o == CO - 1,
                                        )
                                    nc.any.tensor_copy(
                                        out=kT[:, jl, tt0 : tt0 + tn], in_=ps[:, :tn]
                                    )
                                wq_t = wkq.tile([128, CO, 128], F32R, tag="wqg")
                                nc.sync.dma_start(
                                    out=wq_t, in_=wqkv[:, :, j * 128 : (j + 1) * 128].bitcast(F32R)
                                )
                                ps = kqps.tile([128, 512], F32, tag="kqps")
                                for o in range(CO):
                                    _mm(
                                        nc,
                                        ps[:, :NQP],
                                        wq_t[:, o],
                                        xnT[:, o, 0:NQP],
                                        o == 0,
                                        o == CO - 1,
                                    )
                                nc.any.tensor_copy(out=qT[:, jl], in_=ps[:, :NQP])
                            # V for this head group (6 heads = 384 cols)
                            wv_t = wvv.tile([128, CO, 384], F32R, tag="wv")
                            nc.sync.dma_start(
                                out=wv_t,
                                in_=wqkv[:, :, 2 * C + g * 384 : 2 * C + (g + 1) * 384].bitcast(F32R),
                            )
                            for ch, (c0, cn) in enumerate(VCHUNKS):
                                ps = vps.tile([128, 384], F32, tag="vps")
                                for o in range(CO):
                                    _mm(
                                        nc,
                                        ps[:cn],
                                        xnT[:, o, c0 : c0 + cn],
                                        wv_t[:, o],
                                        o == 0,
                                        o == CO - 1,
                                    )
                                vv = vsb[:cn, ch].rearrange("p (j s) -> p j s", s=192)
                                pv = ps[:cn].rearrange("p (j h e) -> p j h e", j=3, h=2)
                                nc.any.tensor_copy(out=vv[:, :, 0:64], in_=pv[:, :, 0])
                                nc.any.tensor_copy(
                                    out=vv[:, :, 128:192], in_=pv[:, :, 1]
                                )

                        # ----- B-g: stage-1 trajectory attention -----
                        with ExitStack() as sb:
                            scps = sb.enter_context(
                                tc.tile_pool(name="scps", bufs=2, space="PSUM")
                            )
                            sxps = sb.enter_context(
                                tc.tile_pool(name="sxps", bufs=2, space="PSUM")
                            )
                            ssps = sb.enter_context(
                                tc.tile_pool(name="ssps", bufs=2, space="PSUM")
                            )
                            expp = sb.enter_context(tc.tile_pool(name="expp", bufs=5))
                            rcpp = sb.enter_context(tc.tile_pool(name="rcpp", bufs=2))
                            for jl, j in enumerate(pairs):
                                for fg in range(4):  # 2 frames per group
                                    ext = []
                                    for hl in range(2):
                                        dlo = hl * 64
                                        exh = []
                                        for fi in range(2):
                                            f = fg * 2 + fi
                                            sps = scps.tile(
                                                [128, 2, 512], F32, tag="sc"
                                            )
                                            for ci in range(2):
                                                c0, cn = VCHUNKS[2 * f + ci]
                                                _mm(
                                                    nc,
                                                    sps[:cn, ci, :Q],
                                                    kT[dlo : dlo + 64, jl, c0 : c0 + cn],
                                                    qT[dlo : dlo + 64, jl, 1:NQ],
                                                )
                                            ex = expp.tile([128, 2, Q], F32R, tag="ex")
                                            nc.scalar.activation(
                                                out=ex,
                                                in_=sps[:, :, :Q],
                                                func=AF.Exp,
                                                scale=SCALE,
                                            )
                                            exh.append(ex)
                                        ext.append(exh)
                                    for fi in range(2):
                                        f = fg * 2 + fi
                                        sx = sxps.tile([128, Q], F32, tag="sx")
                                        ss = ssps.tile([128, Q], F32, tag="ss")
                                        for hl in range(2):
                                            gh = jl * 2 + hl
                                            for ci in range(2):
                                                c0, cn = VCHUNKS[2 * f + ci]
                                                exc = ext[hl][fi][:cn, ci]
                                                first = hl == 0 and ci == 0
                                                last = hl == 1 and ci == 1
                                                _mm(
                                                    nc,
                                                    sx,
                                                    _vseg(vsb, 2 * f + ci, cn, jl, hl),
                                                    exc,
                                                    first,
                                                    last,
                                                )
                                                _mm(
                                                    nc,
                                                    ss,
                                                    _oseg(onez, cn, hl),
                                                    exc,
                                                    first,
                                                    last,
                                                )
                                        rc = rcpp.tile([128, Q], F32, tag="rc")
                                        nc.vector.reciprocal(out=rc, in_=ss)
                                        nc.vector.tensor_tensor(
                                            out=x1T[:, j, f],
                                            in0=sx,
                                            in1=rc,
                                            op=ALU.mult,
                                        )

                        # ----- B5-g: cls attention (over all tokens) -----
                        with ExitStack() as s5:
                            cps = s5.enter_context(
                                tc.tile_pool(name="cps", bufs=2, space="PSUM")
                            )
                            czps = s5.enter_context(
                                tc.tile_pool(name="czps", bufs=2, space="PSUM")
                            )
                            cops = s5.enter_context(
                                tc.tile_pool(name="cops", bufs=2, space="PSUM")
                            )
                            cexp = s5.enter_context(tc.tile_pool(name="cexp", bufs=2))
                            czrp = s5.enter_context(tc.tile_pool(name="czrp", bufs=2))
                            for jl, j in enumerate(pairs):
                                # column 0 = cls query; column 1 = padding (the
                                # first real query) so fp32r dst free counts
                                # stay even
                                co_ps = cops.tile([128, 2], F32, tag="co")
                                zps = czps.tile([128, NCH, 2], F32, tag="cz")
                                zrr = czrp.tile([128, 1], F32, tag="zr")
                                exs = []
                                for hl in range(2):
                                    dlo = hl * 64
                                    sps = cps.tile([128, NCH, 2], F32, tag="cs")
                                    for ch, (c0, cn) in enumerate(VCHUNKS):
                                        _mm(
                                            nc,
                                            sps[:cn, ch],
                                            kT[dlo : dlo + 64, jl, c0 : c0 + cn],
                                            qT[dlo : dlo + 64, jl, 0:2],
                                        )
                                    ex = cexp.tile([128, NCH, 2], F32R, tag="cex")
                                    nc.scalar.activation(
                                        out=ex, in_=sps, func=AF.Exp, scale=SCALE
                                    )
                                    exs.append(ex)
                                # per-chunk partition-restricted sums so junk
                                # lanes (partitions past chunk size) are never read
                                for ch, (c0, cn) in enumerate(VCHUNKS):
                                    for hl in range(2):
                                        _mm(
                                            nc,
                                            zps[:, ch],
                                            _oseg(onez, cn, hl),
                                            exs[hl][:cn, ch],
                                            hl == 0,
                                            hl == 1,
                                        )
                                nc.vector.reduce_sum(
                                    out=zrr,
                                    in_=zps[:, :, 0],
                                    axis=mybir.AxisListType.X,
                                )
                                nc.vector.reciprocal(out=zrr, in_=zrr)
                                for hl in range(2):
                                    gh = jl * 2 + hl
                                    for ch, (c0, cn) in enumerate(VCHUNKS):
                                        _mm(
                                            nc,
                                            co_ps,
                                            _vseg(vsb, ch, cn, jl, hl),
                                            exs[hl][:cn, ch],
                                            hl == 0 and ch == 0,
                                            hl == 1 and ch == NCH - 1,
                                        )
                                nc.vector.tensor_scalar_mul(
                                    outT[:, j, 0:1], co_ps[:, 0:1], zrr
                                )

                # ---------- Phase C: stage-2 frame attention ----------
                with ExitStack() as sC:
                    xdp = sC.enter_context(tc.tile_pool(name="xdp", bufs=1))
                    q2p = sC.enter_context(tc.tile_pool(name="q2p", bufs=1))
                    wsC = sC.enter_context(tc.tile_pool(name="wsC", bufs=2))
                    k2ps = sC.enter_context(
                        tc.tile_pool(name="k2ps", bufs=3, space="PSUM")
                    )
                    lps = sC.enter_context(tc.tile_pool(name="lps", bufs=2, space="PSUM"))
                    prp = sC.enter_context(tc.tile_pool(name="prp", bufs=3))
                    elp = sC.enter_context(tc.tile_pool(name="elp", bufs=2))
                    zzp = sC.enter_context(tc.tile_pool(name="zzp", bufs=2))

                    # x_diag: query i<196 -> frame 0, else frame 1 (frame-permuted)
                    xdT = xdp.tile([128, CO, Q], F32R)
                    nc.any.tensor_copy(out=xdT[:, :, 0:P], in_=x1T[:, :, 0, 0:P])
                    nc.any.tensor_copy(out=xdT[:, :, P:Q], in_=x1T[:, :, 1, P:Q])

                    q2T = q2p.tile([128, CO, Q], F32)
                    for j in range(NPAIR):
                        wq_t = wsC.tile([128, CO, 128], F32R, tag="wqC")
                        nc.sync.dma_start(
                            out=wq_t, in_=wq_fm[:, :, j * 128 : (j + 1) * 128].bitcast(F32R)
                        )
                        ps = k2ps.tile([128, Q], F32, tag="k2")
                        for o in range(CO):
                            _mm(nc, ps, wq_t[:, o], xdT[:, o], o == 0, o == CO - 1)
                        nc.any.tensor_copy(out=q2T[:, j], in_=ps)

                    for j in range(NPAIR):
                        wk_t = wsC.tile([128, CO, 128], F32R, tag="wkC")
                        nc.sync.dma_start(
                            out=wk_t, in_=wk2_fm[:, :, j * 128 : (j + 1) * 128].bitcast(F32R)
                        )
                        el = elp.tile([128, F, Q], F32, tag="el")
                        for f in range(F):
                            ps = k2ps.tile([128, Q], F32, tag="k2")
                            for o in range(CO):
                                _mm(
                                    nc,
                                    ps,
                                    wk_t[:, o],
                                    x1T[:, o, f],
                                    o == 0,
                                    o == CO - 1,
                                )
                            pr = prp.tile([128, Q], F32R, tag="pr")
                            nc.vector.tensor_tensor(
                                out=pr, in0=ps, in1=q2T[:, j], op=ALU.mult
                            )
                            lp = lps.tile([128, Q], F32, tag="lg")
                            _mm(nc, lp, e2, pr)
                            nc.scalar.activation(
                                out=el[:, f], in_=lp, func=AF.Exp, scale=SCALE
                            )
                        # Z = sum_f el[f]: tree split across gpsimd + DVE
                        zz = zzp.tile([128, Q], F32, tag="zz")
                        za = zzp.tile([128, Q], F32, tag="za")
                        zb = zzp.tile([128, Q], F32, tag="zb")
                        nc.gpsimd.tensor_tensor(
                            out=za, in0=el[:, 0], in1=el[:, 1], op=ALU.add
                        )
                        nc.vector.tensor_tensor(
                            out=zb, in0=el[:, 2], in1=el[:, 3], op=ALU.add
                        )
                        nc.gpsimd.tensor_tensor(
                            out=za, in0=za, in1=el[:, 4], op=ALU.add
                        )
                        nc.vector.tensor_tensor(
                            out=zb, in0=zb, in1=el[:, 5], op=ALU.add
                        )
                        nc.gpsimd.tensor_tensor(
                            out=za, in0=za, in1=el[:, 6], op=ALU.add
                        )
                        nc.vector.tensor_tensor(
                            out=zb, in0=zb, in1=el[:, 7], op=ALU.add
                        )
                        nc.vector.tensor_tensor(out=zz, in0=za, in1=zb, op=ALU.add)
                        nc.vector.reciprocal(out=zz, in_=zz)
                        acc = outT[:, j, 1:NQ]
                        nc.vector.tensor_tensor(
                            out=acc, in0=x1T[:, j, 0], in1=el[:, 0], op=ALU.mult
                        )
                        for f in range(1, F):
                            tm = prp.tile([128, Q], F32, tag="tm")
                            # odd-f multiplies ride the otherwise-idle gpsimd
                            eng = nc.gpsimd if f % 2 else nc.vector
                            eng.tensor_tensor(
                                out=tm, in0=x1T[:, j, f], in1=el[:, f], op=ALU.mult
                            )
                            nc.vector.tensor_tensor(
                                out=acc, in0=acc, in1=tm, op=ALU.add
                            )
                        nc.vector.tensor_tensor(out=acc, in0=acc, in1=zz, op=ALU.mult)

        # ---------- Phase D: proj + residual + LN2 + MLP + output ----------
        with ExitStack() as sD:
            wpp = sD.enter_context(tc.tile_pool(name="wpD", bufs=4))
            w2p = sD.enter_context(tc.tile_pool(name="w2D", bufs=3))
            xop = sD.enter_context(tc.tile_pool(name="xoD", bufs=1))
            pjp = sD.enter_context(tc.tile_pool(name="pjD", bufs=1))
            h1p = sD.enter_context(tc.tile_pool(name="h1D", bufs=1))
            m2p = sD.enter_context(tc.tile_pool(name="m2D", bufs=1))
            xn2p = sD.enter_context(tc.tile_pool(name="xn2D", bufs=1))
            gbD = sD.enter_context(tc.tile_pool(name="gbD", bufs=1))
            finp = sD.enter_context(tc.tile_pool(name="finp", bufs=2))
            ln2p = sD.enter_context(tc.tile_pool(name="ln2p", bufs=4))
            mps = sD.enter_context(tc.tile_pool(name="mps", bufs=3, space="PSUM"))
            tps2 = sD.enter_context(tc.tile_pool(name="tps2", bufs=4, space="PSUM"))

            g2f = gbD.tile([128, CO], F32, tag="g2f")
            nc.sync.dma_start(out=g2f, in_=g2_d[:].rearrange("(o p) -> p o", p=128))
            b2f = gbD.tile([128, CO], F32, tag="b2f")
            nc.sync.dma_start(out=b2f, in_=b2_d[:].rearrange("(o p) -> p o", p=128))

            xown = xop.tile([128, 4, C], F32, tag="xo")
            for tt, (t0, pt) in enumerate(TT4):
                nc.sync.dma_start(out=xown[:pt, tt], in_=xp_d[t0 : t0 + pt])

            projT = pjp.tile([128, CO, NQ], F32)
            for j in range(CO):
                wp_t = wpp.tile([128, CO, 128], F32R, tag="wpD")
                nc.sync.dma_start(out=wp_t, in_=wp_fm[:, :, j * 128 : (j + 1) * 128].bitcast(F32R))
                ps = mps.tile([128, 512], F32, tag="mm")
                for o in range(CO):
                    _mm(nc, ps[:, :NQP], wp_t[:, o], outT[:, o], o == 0, o == CO - 1)
                nc.vector.tensor_scalar(
                    projT[:, j], ps[:, :NQ], bp_sb[:, j : j + 1], None, ALU.add
                )

            # residual + LN2 (token-major), then transpose to xn2T
            x2 = xop.tile([128, 4, C], F32, tag="x2")
            xn2T = xn2p.tile([128, CO, NQP], F32R)
            for tt, (t0, pt) in enumerate(TT4):
                for o in range(CO):
                    pst = tps2.tile([128, 128], F32, tag="tp2")
                    nc.tensor.transpose(
                        pst[:pt], projT[:, o, t0 : t0 + pt], ident
                    )
                    nc.vector.tensor_tensor(
                        out=x2[:pt, tt, o * 128 : (o + 1) * 128],
                        in0=pst[:pt],
                        in1=xown[:pt, tt, o * 128 : (o + 1) * 128],
                        op=ALU.add,
                    )
                xn2 = finp.tile([128, C], F32, tag="xn2")
                _layernorm_tile(nc, ln2p, x2[:, tt], pt, eps_t, xn2)
                for o in range(CO):
                    pst = tps2.tile([128, 128], F32, tag="tp2")
                    nc.tensor.transpose(
                        pst[:, :pt], xn2[:pt, o * 128 : (o + 1) * 128], ident[:pt, :pt]
                    )
                    nc.vector.tensor_scalar(
                        xn2T[:, o, t0 : t0 + pt],
                        pst[:, :pt],
                        g2f[:, o : o + 1],
                        b2f[:, o : o + 1],
                        ALU.mult,
                        ALU.add,
                    )

            # MLP up + gelu
            h1T = h1p.tile([128, HO, NQP], F32R)
            for m in range(HO):
                w1_t = wpp.tile([128, CO, 128], F32R, tag="w1D")
                nc.sync.dma_start(out=w1_t, in_=w1_fm[:, :, m * 128 : (m + 1) * 128].bitcast(F32R))
                ps = mps.tile([128, 512], F32, tag="mm")
                for o in range(CO):
                    _mm(nc, ps[:, :NQP], w1_t[:, o], xn2T[:, o], o == 0, o == CO - 1)
                nc.scalar.activation(
                    out=h1T[:, m, :NQ],
                    in_=ps[:, :NQ],
                    func=AF.Gelu,
                    bias=bf1_sb[:, m : m + 1],
                    scale=1.0,
                )

            # MLP down
            m2T = m2p.tile([128, CO, NQ], F32)
            for j in range(CO):
                w2_t = w2p.tile([128, HO, 128], F32R, tag="w2D")
                nc.sync.dma_start(out=w2_t, in_=w2_fm[:, :, j * 128 : (j + 1) * 128].bitcast(F32R))
                ps = mps.tile([128, 512], F32, tag="mm")
                for m in range(HO):
                    _mm(nc, ps[:, :NQP], w2_t[:, m], h1T[:, m], m == 0, m == HO - 1)
                nc.vector.tensor_scalar(
                    m2T[:, j], ps[:, :NQ], bf2_sb[:, j : j + 1], None, ALU.add
                )

            # transpose back to token-major, final residual, store
            for tt, (t0, pt) in enumerate(TT4):
                fo = finp.tile([128, C], F32, tag="fo")
                for o in range(CO):
                    pst = tps2.tile([128, 128], F32, tag="tp2")
                    nc.tensor.transpose(pst[:pt], m2T[:, o, t0 : t0 + pt], ident)
                    nc.vector.tensor_tensor(
                        out=fo[:pt, o * 128 : (o + 1) * 128],
                        in0=pst[:pt],
                        in1=x2[:pt, tt, o * 128 : (o + 1) * 128],
                        op=ALU.add,
                    )
                nc.sync.dma_start(out=out_d[t0 : t0 + pt], in_=fo[:pt])

    nc.compile()
    return nc


_CACHE = {}


def _get_nc():
    if "nc" not in _CACHE:
        _CACHE["nc"] = build_nc()
    return _CACHE["nc"]


def _make_in_maps(inputs):
    f32 = np.float32
    x = np.asarray(inputs["x"], f32)
    wqkv = np.ascontiguousarray(np.asarray(inputs["Wqkv"], f32))
    wq = np.ascontiguousarray(np.asarray(inputs["Wq"], f32))
    wk2 = np.ascontiguousarray(np.asarray(inputs["Wkv"], f32)[:, :C])
    wp = np.ascontiguousarray(np.asarray(inputs["Wp"], f32))
    w1 = np.ascontiguousarray(np.asarray(inputs["W1"], f32))
    w2 = np.ascontiguousarray(np.asarray(inputs["W2"], f32))
    ident = np.eye(128, dtype=f32)
    onez = np.zeros((128, 192), dtype=f32)
    onez[:, 64:128] = 1.0
    e2 = np.zeros((128, 128), dtype=f32)
    e2[:64, :64] = 1.0
    e2[64:, 64:] = 1.0
    common = dict(
        wqkv=wqkv,
        wq=wq,
        wk2=wk2,
        wp=wp,
        w1=w1,
        w2=w2,
        g1=np.asarray(inputs["g1"], f32),
        b1=np.asarray(inputs["b1"], f32),
        g2=np.asarray(inputs["g2"], f32),
        b2=np.asarray(inputs["b2"], f32),
        bp=np.asarray(inputs["bp"], f32),
        bf1=np.asarray(inputs["bf1"], f32),
        bf2=np.asarray(inputs["bf2"], f32),
        ident=ident,
        onez=onez,
        e2=e2,
    )
    in_maps = []
    for c in range(8):
        b, cl = c // 4, c % 4
        f0 = 2 * cl
        order = [(f0 + i) % F for i in range(F)]
        xb = x[b]
        xp = np.concatenate(
            [xb[:1], xb[1:].reshape(F, P, C)[order].reshape(S, C)], axis=0
        )
        m = dict(common)
        m["xp"] = np.ascontiguousarray(xp)
        in_maps.append(m)
    return in_maps


def kernel(**inputs):
    from concourse.bass_utils import run_bass_kernel_spmd

    in_maps = _make_in_maps(inputs)
    res = run_bass_kernel_spmd(_get_nc(), in_maps, core_ids=list(range(8)))
    outs = res.results
    x = np.asarray(inputs["x"])
    full = np.empty((x.shape[0], N, C), dtype=np.float32)
    for c in range(8):
        r = outs[c]["out"]
        b, cl = c // 4, c % 4
        if cl == 0:
            full[b, 0] = r[0]
        full[b, 1 + Q * cl : 1 + Q * (cl + 1)] = r[1:]
    return full
